# revision 1
# baseline (speedup 1.0000x reference)
"""MoE (top-2 of 8 experts) Trainium2 kernel.

Strategy: expert-parallel across the 8 NeuronCores. The (cheap) router runs
on host CPU; the host gathers each expert's routed tokens (already
transposed to [D, C] so the device needs no transposes), each core runs the
dense expert MLP  y = (silu(x @ w1_e) @ w2_e) * combine_weight  for its
expert's tokens only (~T*K/E tokens instead of all T — a 4x compute saving
over the dense formulation), and the host scatter-adds the per-expert
contributions back into the full [B,S,D] output.

Self-contained: only environment packages (numpy/jax/concourse) are used.
"""

import os
import sys

import numpy as np

# concourse ships on sys.path via the container's sitecustomize
# (/root/.axon_site/_ro/trn_rl_repo); /opt copy is a fallback only.
if "/opt/trn_rl_repo" not in sys.path:
    sys.path.append("/opt/trn_rl_repo")

B, S, D_MODEL, D_FF, N_EXPERTS, TOP_K = 2, 2048, 1024, 2048, 8, 2
T = B * S
N_CORES = 8

# compute dtype for the expert MLP matmuls: "bf16", "f32", or "f32r"
COMPUTE_DTYPE = os.environ.get("BASS_MOE_DTYPE", "bf16")

_PROGRAM_CACHE: dict = {}
LAST_BUILD = {}


def _round_up(v: int, m: int) -> int:
    return ((v + m - 1) // m) * m


def _blocks(C: int):
    """Token blocks of <=512 (PSUM-bank limit on matmul free dim)."""
    out = []
    b0 = 0
    while b0 < C:
        bs = min(512, C - b0)
        out.append((b0, bs))
        b0 += bs
    return out


def _build_program(C: int, cdtype: str, repeat: int = 1,
                   timing_only: bool = False):
    """Build + compile the per-core expert-MLP program for capacity C.

    repeat>1 wraps the compute in a device-side loop re-running the same
    work; used only for wall-clock HW timing (results unchanged).
    timing_only=True swaps the big I/O tensors for Internal DRAM scratch
    (garbage data) so per-call host<->device transfer is negligible.
    """
    import contextlib
    import concourse.tile as tile
    from concourse import bacc, mybir

    if cdtype == "bf16":
        mdt = mybir.dt.bfloat16
    elif cdtype == "f32":
        mdt = mybir.dt.float32
    elif cdtype == "f32r":
        mdt = mybir.dt.float32r
    else:
        raise ValueError(cdtype)

    f32 = mybir.dt.float32
    KD = D_MODEL // 128   # 8  k-chunks for matmul 1
    KF = D_FF // 128      # 16 k-chunks for matmul 2

    nc = bacc.Bacc("TRN2", target_bir_lowering=False, debug=False,
                   num_devices=N_CORES)
    ik = "Internal" if timing_only else "ExternalInput"
    ok = "Internal" if timing_only else "ExternalOutput"
    xt_d = nc.dram_tensor("xt", [D_MODEL, C], mdt, kind=ik).ap()
    w1_d = nc.dram_tensor("w1", [D_MODEL, D_FF], mdt, kind=ik).ap()
    w2_d = nc.dram_tensor("w2", [D_FF, D_MODEL], mdt, kind=ik).ap()
    cw_d = nc.dram_tensor("cw", [C], f32, kind=ik).ap()
    y_d = nc.dram_tensor("y", [C, D_MODEL], f32, kind=ok).ap()
    if timing_only:
        tin = nc.dram_tensor("tin", [128, 1], f32, kind="ExternalInput").ap()
        tout = nc.dram_tensor("tout", [128, 1], f32, kind="ExternalOutput").ap()

    silu = mybir.ActivationFunctionType.Silu

    with tile.TileContext(nc) as tc:
        with (
            tc.tile_pool(name="wpool", bufs=1) as wpool,
            tc.tile_pool(name="xpool", bufs=2) as xpool,
            # h tiles live within one block-group pass; bufs=1 keeps the
            # static pool allocation inside SBUF for any capacity C
            tc.tile_pool(name="hpool", bufs=1) as hpool,
            tc.tile_pool(name="cwpool", bufs=3) as cwpool,
            tc.tile_pool(name="ypool", bufs=3) as ypool,
            tc.tile_pool(name="pspool", bufs=1, space="PSUM") as pspool,
        ):
            # resident weights; DMAs split into column chunks and emitted
            # around the first token DMAs so the PE can start ~3us in:
            # w1 cols 0:512 -> (body emits first xt group) -> rest of w1
            # by 512-col chunks -> w2.
            w1_sb = [wpool.tile([128, D_FF], mdt, tag=f"w1_{kk}",
                                name=f"w1t{kk}") for kk in range(KD)]
            w2_sb = [wpool.tile([128, D_MODEL], mdt, tag=f"w2_{kk}",
                                name=f"w2t{kk}") for kk in range(KF)]

            def emit_weight_pre():
                for kk in range(KD):
                    nc.sync.dma_start(w1_sb[kk][:, 0:512],
                                      w1_d[kk * 128:(kk + 1) * 128, 0:512])

            def emit_weight_rest():
                for c0 in range(512, D_FF, 512):
                    for kk in range(KD):
                        nc.sync.dma_start(
                            w1_sb[kk][:, c0:c0 + 512],
                            w1_d[kk * 128:(kk + 1) * 128, c0:c0 + 512])
                for kk in range(KF):
                    nc.sync.dma_start(w2_sb[kk][:],
                                      w2_d[kk * 128:(kk + 1) * 128, :])

            if timing_only:
                tsb = cwpool.tile([128, 1], f32, tag="tsb")
                nc.sync.dma_start(tsb[:], tin[:])
            if repeat > 1:
                # weights resident across iterations; load them up front
                emit_weight_pre()
                emit_weight_rest()
                weight_hook = None
            else:
                weight_hook = (emit_weight_pre, emit_weight_rest)
            rep_ctx = (tc.For_i(0, repeat, 1) if repeat > 1
                       else contextlib.nullcontext())
            with rep_ctx:
                _emit_body(nc, tc, C, mdt, f32, silu, KD, KF,
                           w1_sb, w2_sb, xpool, hpool, cwpool, ypool, pspool,
                           xt_d, cw_d, y_d, weight_hook)
            if timing_only:
                nc.sync.dma_start(tout[:], tsb[:])

    nc.compile()
    return nc


def _emit_body(nc, tc, C, mdt, f32, silu, KD, KF, w1_sb, w2_sb,
               xpool, hpool, cwpool, ypool, pspool, xt_d, cw_d, y_d,
               weight_hook=None):
    """One full pass over the C tokens.

    Token blocks of <=512 are processed in groups of up to GROUP blocks;
    within a group matmul-1 iterates (ff, k) in the outer loops and blocks
    innermost, so each stationary-weight load is reused across the group's
    blocks (LDWEIGHTS count /GROUP on the PE).
    """
    GROUP = 3
    blocks = _blocks(C)
    n_tm_total = C // 128

    # whole combine-weight vector, one DMA: [128, C/128] (token t = col t//128,
    # partition t%128)
    cw_sb = cwpool.tile([128, n_tm_total], f32, tag="cw")
    nc.sync.dma_start(cw_sb[:], cw_d.rearrange("(n p) -> p n", p=128))

    gi = 0
    while gi < len(blocks):
        grp = blocks[gi:gi + GROUP]
        gi += GROUP
        nb = len(grp)

        # stream the group's tokens in (already transposed on host);
        # in the first group, interleave with the weight chunks so the PE
        # can start as soon as xt[b0] + the first w1 columns land
        first_group = weight_hook is not None and gi == len(blocks[:GROUP])
        if first_group:
            weight_hook[0]()  # first w1 column chunk
        xt_sb = {}
        for bi, (b0, bs) in enumerate(grp):
            for kk in range(KD):
                t = xpool.tile([128, bs], mdt, tag=f"x{kk}_{bi}")
                nc.sync.dma_start(
                    t[:], xt_d[kk * 128:(kk + 1) * 128, b0:b0 + bs])
                xt_sb[kk, bi] = t

        if first_group:
            weight_hook[1]()  # rest of the weights, behind the first xt group

        # h[ff, tok] = silu(w1.T-slice @ x) — stored transposed so it can be
        # the stationary operand of matmul 2. Blocks innermost: one w1
        # stationary load serves nb matmuls.
        h_sb = {}
        for fm in range(KF):
            pss = []
            for bi in range(nb):
                ps = pspool.tile([128, grp[bi][1]], f32, tag="ps",
                                 bufs=8, name=f"ps_h{bi}")
                pss.append(ps)
            # first two fm of the first group run block-outer so each chain
            # needs only one block's xt — fills the PE while the rest of the
            # xt group is still in flight
            if first_group and fm < 2:
                for bi in range(nb):
                    for kk in range(KD):
                        nc.tensor.matmul(
                            pss[bi][:],
                            lhsT=w1_sb[kk][:, fm * 128:(fm + 1) * 128],
                            rhs=xt_sb[kk, bi][:],
                            start=(kk == 0), stop=(kk == KD - 1))
            else:
                for kk in range(KD):
                    for bi in range(nb):
                        nc.tensor.matmul(
                            pss[bi][:],
                            lhsT=w1_sb[kk][:, fm * 128:(fm + 1) * 128],
                            rhs=xt_sb[kk, bi][:],
                            start=(kk == 0), stop=(kk == KD - 1))
            for bi in range(nb):
                h = hpool.tile([128, grp[bi][1]], mdt, tag=f"h{fm}_{bi}")
                nc.scalar.activation(h[:], pss[bi][:], silu)
                h_sb[fm, bi] = h

        # y[tok, d] = (h.T @ w2) * cw[tok]
        # fk outer / dn inner: each h stationary load serves both dn tiles.
        # Each 512-wide output half is evacuated + DMA'd independently so the
        # store of half 0 overlaps the accumulation of half 1.
        ND = D_MODEL // 512
        for bi, (b0, bs) in enumerate(grp):
            for tm in range(bs // 128):
                t0 = b0 + tm * 128
                ys = ypool.tile([128, D_MODEL], f32, tag="y")
                pys = []
                for dn in range(ND):
                    py = pspool.tile([128, 512], f32, tag="ps", bufs=8,
                                     name=f"ps_y{dn}")
                    pys.append(py)
                for fk in range(KF):
                    for dn in range(ND):
                        nc.tensor.matmul(
                            pys[dn][:],
                            lhsT=h_sb[fk, bi][:, tm * 128:(tm + 1) * 128],
                            rhs=w2_sb[fk][:, dn * 512:(dn + 1) * 512],
                            start=(fk == 0), stop=(fk == KF - 1))
                for dn in range(ND):
                    # out = psum * combine_weight (per-partition scalar)
                    nc.scalar.mul(ys[:, dn * 512:(dn + 1) * 512], pys[dn][:],
                                  cw_sb[:, t0 // 128:t0 // 128 + 1])
                    nc.sync.dma_start(
                        y_d[t0:t0 + 128, dn * 512:(dn + 1) * 512],
                        ys[:, dn * 512:(dn + 1) * 512])


def _route(x: np.ndarray, gate_w: np.ndarray):
    """Router on host CPU with the reference's exact jax ops/dtypes."""
    try:
        import jax
        import jax.numpy as jnp
        with jax.default_device(jax.devices("cpu")[0]):
            logits = jnp.einsum('bsd,de->bse', jnp.asarray(x),
                                jnp.asarray(gate_w))
            top_logits, top_idx = jax.lax.top_k(logits, TOP_K)
            top_w = jax.nn.softmax(top_logits, axis=-1)
            ti = np.asarray(top_idx).reshape(T, TOP_K)
            tw = np.asarray(top_w).reshape(T, TOP_K).astype(np.float32)
    except Exception:
        # numpy fallback (same selection semantics as jax.lax.top_k)
        logits = (x.reshape(T, D_MODEL) @ gate_w).astype(np.float32)
        i0 = np.argmax(logits, axis=1)
        masked = logits.copy()
        masked[np.arange(T), i0] = -np.inf
        i1 = np.argmax(masked, axis=1)
        v0 = logits[np.arange(T), i0]
        v1 = logits[np.arange(T), i1]
        e1 = np.exp(v1 - v0)
        w0 = 1.0 / (1.0 + e1)
        ti = np.stack([i0, i1], 1)
        tw = np.stack([w0, 1.0 - w0], 1).astype(np.float32)
    return ti, tw


def kernel(x: np.ndarray, gate_w: np.ndarray, w1: np.ndarray,
           w2: np.ndarray) -> np.ndarray:
    from concourse.bass_utils import run_bass_kernel_spmd
    import ml_dtypes

    x = np.asarray(x, dtype=np.float32)
    gate_w = np.asarray(gate_w, dtype=np.float32)
    w1 = np.asarray(w1, dtype=np.float32)
    w2 = np.asarray(w2, dtype=np.float32)

    ti, tw = _route(x, gate_w)

    x2d = x.reshape(T, D_MODEL)
    tokens, weights = [], []
    for e in range(N_EXPERTS):
        rows, ks = np.nonzero(ti == e)
        tokens.append(rows)
        weights.append(tw[rows, ks])
    counts = [len(t) for t in tokens]
    C = _round_up(max(max(counts), 1), 128)

    key = (C, COMPUTE_DTYPE)
    if key not in _PROGRAM_CACHE:
        _PROGRAM_CACHE[key] = _build_program(C, COMPUTE_DTYPE)
    nc = _PROGRAM_CACHE[key]

    np_dt = ml_dtypes.bfloat16 if COMPUTE_DTYPE == "bf16" else np.float32

    in_maps = []
    for e in range(N_EXPERTS):
        n = counts[e]
        xt = np.zeros((D_MODEL, C), dtype=np_dt)
        if n:
            xt[:, :n] = x2d[tokens[e]].astype(np_dt).T
        cw = np.zeros((C,), dtype=np.float32)
        cw[:n] = weights[e]
        in_maps.append({
            "xt": xt,
            "w1": w1[e].astype(np_dt),
            "w2": w2[e].astype(np_dt),
            "cw": cw,
        })

    res = run_bass_kernel_spmd(nc, in_maps, core_ids=list(range(N_CORES)))

    out2d = np.zeros((T, D_MODEL), dtype=np.float32)
    for e in range(N_EXPERTS):
        n = counts[e]
        if n:
            out2d[tokens[e]] += res.results[e]["y"][:n]

    LAST_BUILD["nc"] = nc
    LAST_BUILD["C"] = C
    return out2d.reshape(B, S, D_MODEL)



# revision 2
# speedup vs baseline: 1.2973x; 1.2973x over previous
"""MoE (top-2 of 8 experts) Trainium2 kernel — fp8 DoubleRow version.

Expert-parallel across the 8 NeuronCores. The (cheap) router runs on host
CPU; each core runs one expert's MLP over its routed tokens.

Device math uses fp8(e4m3) matmuls in DoubleRow perf mode (K=256 per
instruction at 0.5 cycles/row — 2x bf16 PE throughput) with a 3-term
residual-compensation scheme to stay well inside the accuracy budget:

    x @ w  ~=  x_hi@w_hi + x_hi@w_lo + x_lo@w_hi

where *_hi = fp8(v) and *_lo = fp8(v - *_hi). Weights are pre-scaled by
2^5 (and x by 2^2) on the host so fp8 subnormals are avoided; the scales
are folded into the silu input scale and the combine weights. The hidden
activation h is split on device: h8 = silu(psum) in fp8 (ACT), h_lo =
hf - h8 (DVE), both feeding matmul 2. Both matmuls keep tokens on the
moving/free dim, so PE cost is exactly 192*C cycles per core.

Self-contained: only environment packages (numpy/jax/concourse/ml_dtypes).
"""

import os
import sys

import numpy as np

# concourse ships on sys.path via the container's sitecustomize
# (/root/.axon_site/_ro/trn_rl_repo); /opt copy is a fallback only.
if "/opt/trn_rl_repo" not in sys.path:
    sys.path.append("/opt/trn_rl_repo")

B, S, D_MODEL, D_FF, N_EXPERTS, TOP_K = 2, 2048, 1024, 2048, 8, 2
T = B * S
N_CORES = 8
KD = D_MODEL // 128   # 8
KF = D_FF // 128      # 16
WS = 32.0             # weight pre-scale (2^5)
XS = 4.0              # x pre-scale (2^2)
NWARM = int(os.environ.get("BASS_MOE_NWARM", "24"))

_PROGRAM_CACHE: dict = {}
LAST_BUILD = {}


def _round_up(v: int, m: int) -> int:
    return ((v + m - 1) // m) * m


def _blocks(C: int):
    """Token blocks: small first block for pipeline spin-up, then <=512."""
    out = []
    b0 = 0
    first = True
    while b0 < C:
        bs = min(256 if first else 512, C - b0)
        out.append((b0, bs))
        b0 += bs
        first = False
    return out


def _build_program(C: int):
    import concourse.tile as tile
    from concourse import bacc, mybir

    f8 = mybir.dt.float8e4
    f32 = mybir.dt.float32
    bf16 = mybir.dt.bfloat16
    DR = mybir.MatmulPerfMode.DoubleRow
    silu = mybir.ActivationFunctionType.Silu
    mult = mybir.AluOpType.mult
    subtract = mybir.AluOpType.subtract

    nc = bacc.Bacc("TRN2", target_bir_lowering=False, debug=False,
                   num_devices=N_CORES)
    # rows (v2, kb8, p): v in (hi, lo)
    xt_d = nc.dram_tensor("xt", [2 * KD * 128, C], f8, kind="ExternalInput").ap()
    # rows (kb8, v2, p): v in (hi, lo)
    w1_d = nc.dram_tensor("w1", [KD * 2 * 128, D_FF], f8, kind="ExternalInput").ap()
    # rows (fb16, v2, p)
    w2_d = nc.dram_tensor("w2", [KF * 2 * 128, D_MODEL], f8, kind="ExternalInput").ap()
    cw_d = nc.dram_tensor("cw", [128, C], f32, kind="ExternalInput").ap()
    y_d = nc.dram_tensor("y", [D_MODEL, C], bf16, kind="ExternalOutput").ap()

    x_re = xt_d.rearrange("(v kb p) c -> p v kb c", p=128, kb=KD)
    w1_re = w1_d.rearrange("(kb v p) f -> p kb v f", p=128, v=2)
    w2_re = w2_d.rearrange("(fb v p) d -> p fb v d", p=128, v=2)
    y_re = y_d.rearrange("(dn p) c -> p dn c", p=128)

    blocks = _blocks(C)
    nb = len(blocks)

    with tile.TileContext(nc) as tc:
        with (
            tc.tile_pool(name="big", bufs=1) as big,
            tc.tile_pool(name="hfpool", bufs=3) as hfpool,
            tc.tile_pool(name="ypool", bufs=2) as ypool,
            tc.tile_pool(name="psh", bufs=3, space="PSUM") as pshpool,
            tc.tile_pool(name="psy", bufs=5, space="PSUM") as psypool,
        ):
            x_sb = big.tile([128, 2, KD, C], f8, name="x_sb")
            w1_sb = big.tile([128, KD, 2, D_FF], f8, name="w1_sb")
            w2_sb = big.tile([128, KF, 2, D_MODEL], f8, name="w2_sb")
            h_sb = big.tile([128, KF, 2, C], f8, name="h_sb")
            cw_sb = big.tile([128, C], f32, name="cw_sb")
            warm = big.tile([128, 2, 512], f8, name="warm")

            # PE warmup: ramp the p-state while input DMAs stream.
            nc.vector.memset(warm[:], 0)
            for i in range(NWARM):
                wps = pshpool.tile([128, 512], f32, tag="psh", name=f"wps{i}")
                nc.tensor.matmul(wps[:], lhsT=warm[:, :, 0:128], rhs=warm[:],
                                 start=True, stop=True, perf_mode=DR)

            # ---- input DMAs, interleaved roughly in consumption order
            def dma_x(bi):
                b0, bs = blocks[bi]
                nc.sync.dma_start(x_sb[:, :, :, b0:b0 + bs],
                                  x_re[:, :, :, b0:b0 + bs])

            dma_x(0)
            nc.sync.dma_start(w1_sb[:, :, :, 0:512], w1_re[:, :, :, 0:512])
            nc.sync.dma_start(w1_sb[:, :, :, 512:1024],
                              w1_re[:, :, :, 512:1024])
            if nb > 1:
                dma_x(1)
            nc.sync.dma_start(w1_sb[:, :, :, 1024:1536],
                              w1_re[:, :, :, 1024:1536])
            nc.sync.dma_start(w1_sb[:, :, :, 1536:2048],
                              w1_re[:, :, :, 1536:2048])
            nc.sync.dma_start(cw_sb[:], cw_d[:])
            nc.sync.dma_start(w2_sb[:, :, :, 0:512], w2_re[:, :, :, 0:512])
            for bi in range(2, nb):
                dma_x(bi)
            nc.sync.dma_start(w2_sb[:, :, :, 512:1024],
                              w2_re[:, :, :, 512:1024])

            def mm1(bi):
                """z = 3-term x@w1 ; h8 = silu fp8 ; hf = silu f32 (per fm)."""
                b0, bs = blocks[bi]
                hfs = []
                for fm in range(KF):
                    ps = pshpool.tile([128, bs], f32, tag="psh",
                                      name=f"psh{bi}_{fm}")
                    n_i = KD + KD // 2
                    i = 0
                    fsl = slice(fm * 128, (fm + 1) * 128)
                    for kb in range(KD):  # P1: (x_hi,x_hi)x(w_hi,w_lo)
                        nc.tensor.matmul(
                            ps[:],
                            lhsT=w1_sb[:, kb, :, fsl],
                            rhs=x_sb[:, 0, kb, b0:b0 + bs].unsqueeze(1)
                                .broadcast_to([128, 2, bs]),
                            start=(i == 0), stop=(i == n_i - 1), perf_mode=DR)
                        i += 1
                    for kb2 in range(0, KD, 2):  # P2: (x_lo,x_lo)x(w_hi,w_hi)
                        nc.tensor.matmul(
                            ps[:],
                            lhsT=w1_sb[:, kb2:kb2 + 2, 0, fsl],
                            rhs=x_sb[:, 1, kb2:kb2 + 2, b0:b0 + bs],
                            start=(i == 0), stop=(i == n_i - 1), perf_mode=DR)
                        i += 1
                    nc.scalar.activation(h_sb[:, fm, 0, b0:b0 + bs], ps[:],
                                         silu, scale=1.0 / (WS * XS))
                    hf = hfpool.tile([128, bs], f32, tag="hf",
                                     name=f"hf{bi}_{fm}")
                    nc.scalar.activation(hf[:], ps[:], silu,
                                         scale=1.0 / (WS * XS))
                    hfs.append(hf)
                return hfs

            def h_lo(bi, hfs):
                b0, bs = blocks[bi]
                for fm in range(KF):
                    nc.vector.scalar_tensor_tensor(
                        h_sb[:, fm, 1, b0:b0 + bs], hfs[fm][:], 1.0,
                        h_sb[:, fm, 0, b0:b0 + bs],
                        op0=mult, op1=subtract)

            def mm2(bi):
                """y = (3-term h@w2) * cw / WS ; one block DMA out."""
                b0, bs = blocks[bi]
                ys = ypool.tile([128, KD, bs], bf16, tag="y", name=f"y{bi}")
                for dn in range(KD):
                    ps = psypool.tile([128, bs], f32, tag="psy",
                                      name=f"psy{bi}_{dn}")
                    n_i = KF + KF // 2
                    i = 0
                    dsl = slice(dn * 128, (dn + 1) * 128)
                    for fb in range(KF):  # P1': (h8,h_lo)x(w2_hi,w2_hi)
                        nc.tensor.matmul(
                            ps[:],
                            lhsT=w2_sb[:, fb, 0, dsl].unsqueeze(1)
                                .broadcast_to([128, 2, 128]),
                            rhs=h_sb[:, fb, :, b0:b0 + bs],
                            start=(i == 0), stop=(i == n_i - 1), perf_mode=DR)
                        i += 1
                    for fb2 in range(0, KF, 2):  # P2': (h8,h8)x(w2_lo,w2_lo)
                        nc.tensor.matmul(
                            ps[:],
                            lhsT=w2_sb[:, fb2:fb2 + 2, 1, dsl],
                            rhs=h_sb[:, fb2:fb2 + 2, 0, b0:b0 + bs],
                            start=(i == 0), stop=(i == n_i - 1), perf_mode=DR)
                        i += 1
                    nc.vector.scalar_tensor_tensor(
                        ys[:, dn, :], ps[:], 1.0 / WS, cw_sb[:, b0:b0 + bs],
                        op0=mult, op1=mult)
                nc.gpsimd.dma_start(y_re[:, :, b0:b0 + bs], ys[:])

            # ---- software-pipelined emission: mm1 one block ahead of mm2
            hfs = mm1(0)
            h_lo(0, hfs)
            for bi in range(1, nb):
                hfs = mm1(bi)
                mm2(bi - 1)
                h_lo(bi, hfs)
            mm2(nb - 1)

    nc.compile()
    return nc


def _route(x: np.ndarray, gate_w: np.ndarray):
    """Router on host CPU with the reference's exact jax ops/dtypes."""
    try:
        import jax
        import jax.numpy as jnp
        with jax.default_device(jax.devices("cpu")[0]):
            logits = jnp.einsum('bsd,de->bse', jnp.asarray(x),
                                jnp.asarray(gate_w))
            top_logits, top_idx = jax.lax.top_k(logits, TOP_K)
            top_w = jax.nn.softmax(top_logits, axis=-1)
            ti = np.asarray(top_idx).reshape(T, TOP_K)
            tw = np.asarray(top_w).reshape(T, TOP_K).astype(np.float32)
    except Exception:
        # numpy fallback (same selection semantics as jax.lax.top_k)
        logits = (x.reshape(T, D_MODEL) @ gate_w).astype(np.float32)
        i0 = np.argmax(logits, axis=1)
        masked = logits.copy()
        masked[np.arange(T), i0] = -np.inf
        i1 = np.argmax(masked, axis=1)
        v0 = logits[np.arange(T), i0]
        v1 = logits[np.arange(T), i1]
        e1 = np.exp(v1 - v0)
        w0 = 1.0 / (1.0 + e1)
        ti = np.stack([i0, i1], 1)
        tw = np.stack([w0, 1.0 - w0], 1).astype(np.float32)
    return ti, tw


def _hi_lo(a: np.ndarray, F8):
    hi = a.astype(F8)
    lo = (a - hi.astype(np.float32)).astype(F8)
    return hi, lo


def kernel(x: np.ndarray, gate_w: np.ndarray, w1: np.ndarray,
           w2: np.ndarray) -> np.ndarray:
    from concourse.bass_utils import run_bass_kernel_spmd
    import ml_dtypes

    F8 = ml_dtypes.float8_e4m3

    x = np.asarray(x, dtype=np.float32)
    gate_w = np.asarray(gate_w, dtype=np.float32)
    w1 = np.asarray(w1, dtype=np.float32)
    w2 = np.asarray(w2, dtype=np.float32)

    ti, tw = _route(x, gate_w)

    x2d = x.reshape(T, D_MODEL)
    tokens, weights = [], []
    for e in range(N_EXPERTS):
        rows, ks = np.nonzero(ti == e)
        tokens.append(rows)
        weights.append(tw[rows, ks])
    counts = [len(t) for t in tokens]
    C = _round_up(max(max(counts), 512), 32)

    if C not in _PROGRAM_CACHE:
        _PROGRAM_CACHE[C] = _build_program(C)
    nc = _PROGRAM_CACHE[C]

    in_maps = []
    for e in range(N_EXPERTS):
        n = counts[e]
        # x: [D, C] scaled by XS, hi/lo split, packed rows (v, kb*128+p)
        xt = np.zeros((D_MODEL, C), dtype=np.float32)
        if n:
            xt[:, :n] = x2d[tokens[e]].T * XS
        x_hi, x_lo = _hi_lo(xt, F8)
        x_pk = np.concatenate([x_hi, x_lo], axis=0)  # [2*D, C]

        w1_hi, w1_lo = _hi_lo(w1[e] * WS, F8)        # [D, DFF]
        # rows (kb, v, p): interleave hi/lo per 128-row block
        w1_pk = np.stack([w1_hi.reshape(KD, 128, D_FF),
                          w1_lo.reshape(KD, 128, D_FF)], axis=1
                         ).reshape(KD * 2 * 128, D_FF)

        w2_hi, w2_lo = _hi_lo(w2[e] * WS, F8)        # [DFF, D]
        w2_pk = np.stack([w2_hi.reshape(KF, 128, D_MODEL),
                          w2_lo.reshape(KF, 128, D_MODEL)], axis=1
                         ).reshape(KF * 2 * 128, D_MODEL)

        cw = np.zeros((C,), dtype=np.float32)
        cw[:n] = weights[e]
        cw_bc = np.broadcast_to(cw[None, :], (128, C)).copy()

        in_maps.append({
            "xt": x_pk,
            "w1": w1_pk,
            "w2": w2_pk,
            "cw": cw_bc,
        })

    res = run_bass_kernel_spmd(nc, in_maps, core_ids=list(range(N_CORES)))

    out2d = np.zeros((T, D_MODEL), dtype=np.float32)
    for e in range(N_EXPERTS):
        n = counts[e]
        if n:
            out2d[tokens[e]] += res.results[e]["y"].astype(np.float32).T[:n]

    LAST_BUILD["nc"] = nc
    LAST_BUILD["C"] = C
    return out2d.reshape(B, S, D_MODEL)


# revision 7
# speedup vs baseline: 1.2991x; 1.0014x over previous
"""MoE (top-2 of 8 experts) Trainium2 kernel — fp8 DoubleRow version.

Expert-parallel across the 8 NeuronCores. The (cheap) router runs on host
CPU; each core runs one expert's MLP over its routed tokens.

Device math uses fp8(e4m3) matmuls in DoubleRow perf mode (K=256 per
instruction at 0.5 cycles/row — 2x bf16 PE throughput) with a 3-term
residual-compensation scheme to stay well inside the accuracy budget:

    x @ w  ~=  x_hi@w_hi + x_hi@w_lo + x_lo@w_hi

where *_hi = fp8(v) and *_lo = fp8(v - *_hi). Weights are pre-scaled by
2^5 (and x by 2^2) on the host so fp8 subnormals are avoided; the scales
are folded into the silu input scale and the combine weights. The hidden
activation h is split on device: h8 = silu(psum) in fp8 (ACT), h_lo =
hf - h8 (DVE), both feeding matmul 2. Both matmuls keep tokens on the
moving/free dim, so PE cost is exactly 192*C cycles per core.

Self-contained: only environment packages (numpy/jax/concourse/ml_dtypes).
"""

import os
import sys

import numpy as np

# concourse ships on sys.path via the container's sitecustomize
# (/root/.axon_site/_ro/trn_rl_repo); /opt copy is a fallback only.
if "/opt/trn_rl_repo" not in sys.path:
    sys.path.append("/opt/trn_rl_repo")

B, S, D_MODEL, D_FF, N_EXPERTS, TOP_K = 2, 2048, 1024, 2048, 8, 2
T = B * S
N_CORES = 8
KD = D_MODEL // 128   # 8
KF = D_FF // 128      # 16
WS = 32.0             # weight pre-scale (2^5)
XS = 4.0              # x pre-scale (2^2)
NWARM = int(os.environ.get("BASS_MOE_NWARM", "24"))

_PROGRAM_CACHE: dict = {}
LAST_BUILD = {}


def _round_up(v: int, m: int) -> int:
    return ((v + m - 1) // m) * m


def _blocks(C: int):
    """Token blocks of <=512; any remainder lands in the last (small) block
    so the post-PE tail is short."""
    out = []
    b0 = 0
    while b0 < C:
        bs = min(512, C - b0)
        out.append((b0, bs))
        b0 += bs
    return out


def _build_program(C: int):
    import concourse.tile as tile
    from concourse import bacc, mybir

    f8 = mybir.dt.float8e4
    f32 = mybir.dt.float32
    bf16 = mybir.dt.bfloat16
    DR = mybir.MatmulPerfMode.DoubleRow
    silu = mybir.ActivationFunctionType.Silu
    mult = mybir.AluOpType.mult
    subtract = mybir.AluOpType.subtract

    nc = bacc.Bacc("TRN2", target_bir_lowering=False, debug=False,
                   num_devices=N_CORES)
    # rows (v2, kb8, p): v in (hi, lo)
    xt_d = nc.dram_tensor("xt", [2 * KD * 128, C], f8, kind="ExternalInput").ap()
    # rows (kb8, v2, p): v in (hi, lo)
    w1_d = nc.dram_tensor("w1", [KD * 2 * 128, D_FF], f8, kind="ExternalInput").ap()
    # rows (fb16, v2, p)
    w2_d = nc.dram_tensor("w2", [KF * 2 * 128, D_MODEL], f8, kind="ExternalInput").ap()
    cw_d = nc.dram_tensor("cw", [128, C], f32, kind="ExternalInput").ap()
    y_d = nc.dram_tensor("y", [D_MODEL, C], bf16, kind="ExternalOutput").ap()

    x_re = xt_d.rearrange("(v kb p) c -> p v kb c", p=128, kb=KD)
    w1_re = w1_d.rearrange("(kb v p) f -> p kb v f", p=128, v=2)
    w2_re = w2_d.rearrange("(fb v p) d -> p fb v d", p=128, v=2)
    y_re = y_d.rearrange("(dn p) c -> p dn c", p=128)

    blocks = _blocks(C)
    nb = len(blocks)

    with tile.TileContext(nc) as tc:
        with (
            tc.tile_pool(name="big", bufs=1) as big,
            tc.tile_pool(name="hfpool", bufs=3) as hfpool,
            tc.tile_pool(name="ypool", bufs=3) as ypool,
            tc.tile_pool(name="psh", bufs=3, space="PSUM") as pshpool,
            tc.tile_pool(name="psy", bufs=5, space="PSUM") as psypool,
        ):
            x_sb = big.tile([128, 2, KD, C], f8, name="x_sb")
            w1_sb = big.tile([128, KD, 2, D_FF], f8, name="w1_sb")
            w2_sb = big.tile([128, KF, 2, D_MODEL], f8, name="w2_sb")
            h_sb = big.tile([128, KF, 2, C], f8, name="h_sb")
            cw_sb = big.tile([128, C], f32, name="cw_sb")
            warm = big.tile([128, 2, 512], f8, name="warm")

            # PE warmup: ramp the p-state while input DMAs stream. Garbage
            # SBUF bytes can decode as fp8 NaN/Inf and wedge the exec unit,
            # so zero the tile first (Pool dispatches fastest).
            nc.gpsimd.memset(warm[:], 0)
            for i in range(NWARM):
                wps = pshpool.tile([128, 512], f32, tag="psh", name=f"wps{i}")
                nc.tensor.matmul(wps[:], lhsT=warm[:, :, 0:128], rhs=warm[:],
                                 start=True, stop=True, perf_mode=DR)

            # ---- input DMAs, interleaved roughly in consumption order
            def dma_x(bi):
                b0, bs = blocks[bi]
                nc.sync.dma_start(x_sb[:, :, :, b0:b0 + bs],
                                  x_re[:, :, :, b0:b0 + bs])

            dma_x(0)
            nc.sync.dma_start(w1_sb[:, :, :, 0:512], w1_re[:, :, :, 0:512])
            nc.sync.dma_start(w1_sb[:, :, :, 512:1024],
                              w1_re[:, :, :, 512:1024])
            if nb > 1:
                dma_x(1)
            nc.sync.dma_start(w1_sb[:, :, :, 1024:1536],
                              w1_re[:, :, :, 1024:1536])
            nc.sync.dma_start(w1_sb[:, :, :, 1536:2048],
                              w1_re[:, :, :, 1536:2048])
            nc.sync.dma_start(cw_sb[:], cw_d[:])
            nc.sync.dma_start(w2_sb[:, :, :, 0:512], w2_re[:, :, :, 0:512])
            for bi in range(2, nb):
                dma_x(bi)
            nc.sync.dma_start(w2_sb[:, :, :, 512:1024],
                              w2_re[:, :, :, 512:1024])

            def mm1(bi):
                """z = 3-term x@w1 ; h8 = silu fp8 ; hf = silu f32 (per fm)."""
                b0, bs = blocks[bi]
                hfs = []
                for fm in range(KF):
                    ps = pshpool.tile([128, bs], f32, tag="psh",
                                      name=f"psh{bi}_{fm}")
                    n_i = KD + KD // 2
                    i = 0
                    fsl = slice(fm * 128, (fm + 1) * 128)
                    for kb in range(KD):  # P1: (x_hi,x_hi)x(w_hi,w_lo)
                        nc.tensor.matmul(
                            ps[:],
                            lhsT=w1_sb[:, kb, :, fsl],
                            rhs=x_sb[:, 0, kb, b0:b0 + bs].unsqueeze(1)
                                .broadcast_to([128, 2, bs]),
                            start=(i == 0), stop=(i == n_i - 1), perf_mode=DR)
                        i += 1
                    for kb2 in range(0, KD, 2):  # P2: (x_lo,x_lo)x(w_hi,w_hi)
                        nc.tensor.matmul(
                            ps[:],
                            lhsT=w1_sb[:, kb2:kb2 + 2, 0, fsl],
                            rhs=x_sb[:, 1, kb2:kb2 + 2, b0:b0 + bs],
                            start=(i == 0), stop=(i == n_i - 1), perf_mode=DR)
                        i += 1
                    nc.scalar.activation(h_sb[:, fm, 0, b0:b0 + bs], ps[:],
                                         silu, scale=1.0 / (WS * XS))
                    hf = hfpool.tile([128, bs], f32, tag="hf",
                                     name=f"hf{bi}_{fm}")
                    nc.scalar.activation(hf[:], ps[:], silu,
                                         scale=1.0 / (WS * XS))
                    hfs.append(hf)
                return hfs

            def h_lo(bi, hfs):
                b0, bs = blocks[bi]
                for fm in range(KF):
                    nc.vector.scalar_tensor_tensor(
                        h_sb[:, fm, 1, b0:b0 + bs], hfs[fm][:], 1.0,
                        h_sb[:, fm, 0, b0:b0 + bs],
                        op0=mult, op1=subtract)

            def mm2(bi):
                """y = (3-term h@w2) * cw / WS ; per-dn DMA out (overlaps)."""
                b0, bs = blocks[bi]
                for dn in range(KD):
                    ps = psypool.tile([128, bs], f32, tag="psy",
                                      name=f"psy{bi}_{dn}")
                    n_i = KF + KF // 2
                    i = 0
                    dsl = slice(dn * 128, (dn + 1) * 128)
                    for fb in range(KF):  # P1': (h8,h_lo)x(w2_hi,w2_hi)
                        nc.tensor.matmul(
                            ps[:],
                            lhsT=w2_sb[:, fb, 0, dsl].unsqueeze(1)
                                .broadcast_to([128, 2, 128]),
                            rhs=h_sb[:, fb, :, b0:b0 + bs],
                            start=(i == 0), stop=(i == n_i - 1), perf_mode=DR)
                        i += 1
                    for fb2 in range(0, KF, 2):  # P2': (h8,h8)x(w2_lo,w2_lo)
                        nc.tensor.matmul(
                            ps[:],
                            lhsT=w2_sb[:, fb2:fb2 + 2, 1, dsl],
                            rhs=h_sb[:, fb2:fb2 + 2, 0, b0:b0 + bs],
                            start=(i == 0), stop=(i == n_i - 1), perf_mode=DR)
                        i += 1
                    ys = ypool.tile([128, bs], bf16, tag="y",
                                    name=f"y{bi}_{dn}")
                    nc.vector.scalar_tensor_tensor(
                        ys[:], ps[:], 1.0 / WS, cw_sb[:, b0:b0 + bs],
                        op0=mult, op1=mult)
                    nc.gpsimd.dma_start(y_re[:, dn, b0:b0 + bs], ys[:])

            # ---- software-pipelined emission: mm1 one block ahead of mm2
            hfs = mm1(0)
            h_lo(0, hfs)
            for bi in range(1, nb):
                hfs = mm1(bi)
                mm2(bi - 1)
                h_lo(bi, hfs)
            mm2(nb - 1)

    nc.compile()
    return nc


def _route(x: np.ndarray, gate_w: np.ndarray):
    """Router on host CPU with the reference's exact jax ops/dtypes."""
    try:
        import jax
        import jax.numpy as jnp
        with jax.default_device(jax.devices("cpu")[0]):
            logits = jnp.einsum('bsd,de->bse', jnp.asarray(x),
                                jnp.asarray(gate_w))
            top_logits, top_idx = jax.lax.top_k(logits, TOP_K)
            top_w = jax.nn.softmax(top_logits, axis=-1)
            ti = np.asarray(top_idx).reshape(T, TOP_K)
            tw = np.asarray(top_w).reshape(T, TOP_K).astype(np.float32)
    except Exception:
        # numpy fallback (same selection semantics as jax.lax.top_k)
        logits = (x.reshape(T, D_MODEL) @ gate_w).astype(np.float32)
        i0 = np.argmax(logits, axis=1)
        masked = logits.copy()
        masked[np.arange(T), i0] = -np.inf
        i1 = np.argmax(masked, axis=1)
        v0 = logits[np.arange(T), i0]
        v1 = logits[np.arange(T), i1]
        e1 = np.exp(v1 - v0)
        w0 = 1.0 / (1.0 + e1)
        ti = np.stack([i0, i1], 1)
        tw = np.stack([w0, 1.0 - w0], 1).astype(np.float32)
    return ti, tw


def _hi_lo(a: np.ndarray, F8):
    hi = a.astype(F8)
    lo = (a - hi.astype(np.float32)).astype(F8)
    return hi, lo


def kernel(x: np.ndarray, gate_w: np.ndarray, w1: np.ndarray,
           w2: np.ndarray) -> np.ndarray:
    from concourse.bass_utils import run_bass_kernel_spmd
    import ml_dtypes

    F8 = ml_dtypes.float8_e4m3

    x = np.asarray(x, dtype=np.float32)
    gate_w = np.asarray(gate_w, dtype=np.float32)
    w1 = np.asarray(w1, dtype=np.float32)
    w2 = np.asarray(w2, dtype=np.float32)

    ti, tw = _route(x, gate_w)

    x2d = x.reshape(T, D_MODEL)
    tokens, weights = [], []
    for e in range(N_EXPERTS):
        rows, ks = np.nonzero(ti == e)
        tokens.append(rows)
        weights.append(tw[rows, ks])
    counts = [len(t) for t in tokens]
    C = _round_up(max(max(counts), 512), 32)

    if C not in _PROGRAM_CACHE:
        _PROGRAM_CACHE[C] = _build_program(C)
    nc = _PROGRAM_CACHE[C]

    in_maps = []
    for e in range(N_EXPERTS):
        n = counts[e]
        # x: [D, C] scaled by XS, hi/lo split, packed rows (v, kb*128+p)
        xt = np.zeros((D_MODEL, C), dtype=np.float32)
        if n:
            xt[:, :n] = x2d[tokens[e]].T * XS
        x_hi, x_lo = _hi_lo(xt, F8)
        x_pk = np.concatenate([x_hi, x_lo], axis=0)  # [2*D, C]

        w1_hi, w1_lo = _hi_lo(w1[e] * WS, F8)        # [D, DFF]
        # rows (kb, v, p): interleave hi/lo per 128-row block
        w1_pk = np.stack([w1_hi.reshape(KD, 128, D_FF),
                          w1_lo.reshape(KD, 128, D_FF)], axis=1
                         ).reshape(KD * 2 * 128, D_FF)

        w2_hi, w2_lo = _hi_lo(w2[e] * WS, F8)        # [DFF, D]
        w2_pk = np.stack([w2_hi.reshape(KF, 128, D_MODEL),
                          w2_lo.reshape(KF, 128, D_MODEL)], axis=1
                         ).reshape(KF * 2 * 128, D_MODEL)

        cw = np.zeros((C,), dtype=np.float32)
        cw[:n] = weights[e]
        cw_bc = np.broadcast_to(cw[None, :], (128, C)).copy()

        in_maps.append({
            "xt": x_pk,
            "w1": w1_pk,
            "w2": w2_pk,
            "cw": cw_bc,
        })

    res = run_bass_kernel_spmd(nc, in_maps, core_ids=list(range(N_CORES)))

    out2d = np.zeros((T, D_MODEL), dtype=np.float32)
    for e in range(N_EXPERTS):
        n = counts[e]
        if n:
            out2d[tokens[e]] += res.results[e]["y"].astype(np.float32).T[:n]

    LAST_BUILD["nc"] = nc
    LAST_BUILD["C"] = C
    return out2d.reshape(B, S, D_MODEL)


# revision 15
# speedup vs baseline: 1.3697x; 1.0543x over previous
"""MoE (top-2 of 8 experts) Trainium2 kernel — fp8 DoubleRow version.

Expert-parallel across the 8 NeuronCores. The (cheap) router runs on host
CPU; each core runs one expert's MLP over its routed tokens.

Device math uses fp8(e4m3) matmuls in DoubleRow perf mode (K=256 per
instruction at 0.5 cycles/row — 2x bf16 PE throughput) with a 3-term
residual-compensation scheme to stay well inside the accuracy budget:

    x @ w  ~=  x_hi@w_hi + x_hi@w_lo + x_lo@w_hi

where *_hi = fp8(v) and *_lo = fp8(v - *_hi). Weights are pre-scaled by
2^5 (and x by 2^2) on the host so fp8 subnormals are avoided; the scales
are folded into the silu input scale and the combine weights. The hidden
activation h is split on device: h8 = silu(psum) in fp8 (ACT), h_lo =
hf - h8 (DVE), both feeding matmul 2. Both matmuls keep tokens on the
moving/free dim, so PE cost is exactly 192*C cycles per core.

Self-contained: only environment packages (numpy/jax/concourse/ml_dtypes).
"""

import os
import sys

import numpy as np

# concourse ships on sys.path via the container's sitecustomize
# (/root/.axon_site/_ro/trn_rl_repo); /opt copy is a fallback only.
if "/opt/trn_rl_repo" not in sys.path:
    sys.path.append("/opt/trn_rl_repo")

B, S, D_MODEL, D_FF, N_EXPERTS, TOP_K = 2, 2048, 1024, 2048, 8, 2
T = B * S
N_CORES = 8
KD = D_MODEL // 128   # 8
KF = D_FF // 128      # 16
WS = 32.0             # weight pre-scale (2^5)
XS = 4.0              # x pre-scale (2^2)
NWARM = int(os.environ.get("BASS_MOE_NWARM", "24"))

_PROGRAM_CACHE: dict = {}
LAST_BUILD = {}


def _round_up(v: int, m: int) -> int:
    return ((v + m - 1) // m) * m


def _blocks(C: int):
    """Token blocks: 320 first (so the x(b0) DMA lands before the first w1
    chunk), then 512s; the remainder lands in the last (small) block so the
    post-PE tail is short."""
    out = []
    b0 = 0
    while b0 < C:
        bs = min(320 if b0 == 0 and C > 512 else 512, C - b0)
        out.append((b0, bs))
        b0 += bs
    return out


def _build_program(C: int):
    import concourse.tile as tile
    from concourse import bacc, mybir

    f8 = mybir.dt.float8e4
    f32 = mybir.dt.float32
    bf16 = mybir.dt.bfloat16
    DR = mybir.MatmulPerfMode.DoubleRow
    silu = mybir.ActivationFunctionType.Silu
    mult = mybir.AluOpType.mult
    subtract = mybir.AluOpType.subtract

    nc = bacc.Bacc("TRN2", target_bir_lowering=False, debug=False,
                   num_devices=N_CORES)
    blocks = _blocks(C)
    nb = len(blocks)

    # All inputs are packed on the host so every DMA chunk is
    # partition-contiguous (>=512B descriptor runs -> full DMA rate) at
    # exactly the granularity the PE consumes:
    #   w1: row fm*128+p holds [kb8, v2, f128]  -> 16 per-fm chunks
    #   w2: row dn*128+p holds [fb16, v2, d128] ->  8 per-dn chunks
    #   x : one tensor per block, row p holds [v2, kb8, bs]
    w1_d = nc.dram_tensor("w1", [KF * 128, KD * 2 * 128], f8,
                          kind="ExternalInput").ap()
    w2_d = nc.dram_tensor("w2", [KD * 128, KF * 2 * 128], f8,
                          kind="ExternalInput").ap()
    x_ds = [nc.dram_tensor(f"xt{bi}", [128, 2 * KD * bs], f8,
                           kind="ExternalInput").ap()
            for bi, (b0, bs) in enumerate(blocks)]
    cw_d = nc.dram_tensor("cw", [128, C], f32, kind="ExternalInput").ap()
    y_d = nc.dram_tensor("y", [D_MODEL, C], bf16, kind="ExternalOutput").ap()

    y_re = y_d.rearrange("(dn p) c -> p dn c", p=128)

    with tile.TileContext(nc) as tc:
        with (
            tc.tile_pool(name="big", bufs=1) as big,
            tc.tile_pool(name="hfpool", bufs=3) as hfpool,
            tc.tile_pool(name="ypool", bufs=3) as ypool,
            tc.tile_pool(name="psh", bufs=3, space="PSUM") as pshpool,
            tc.tile_pool(name="psy", bufs=5, space="PSUM") as psypool,
        ):
            x_sbs = [big.tile([128, 2, KD, bs], f8, name=f"x_sb{bi}")
                     for bi, (b0, bs) in enumerate(blocks)]
            w1_sb = big.tile([128, KF, KD, 2, 128], f8, name="w1_sb")
            w2_sb = big.tile([128, KD, KF, 2, 128], f8, name="w2_sb")
            h_sb = big.tile([128, KF, 2, C], f8, name="h_sb")
            cw_sb = big.tile([128, C], f32, name="cw_sb")
            warm = big.tile([128, 2, 256], f8, name="warm")

            # PE warmup: ramp the p-state while input DMAs stream. Garbage
            # SBUF bytes can decode as fp8 NaN/Inf and wedge the exec unit,
            # so zero the tile first (Pool dispatches fastest).
            nc.gpsimd.memset(warm[:], 0)
            for i in range(NWARM):
                wps = pshpool.tile([128, 128], f32, tag="psh", name=f"wps{i}")
                nc.tensor.matmul(wps[:], lhsT=warm[:, :, 0:128],
                                 rhs=warm[:, :, 0:128],
                                 start=True, stop=True, perf_mode=DR)

            # ---- input DMAs, in consumption order
            def dma_x(bi):
                b0, bs = blocks[bi]
                nc.sync.dma_start(
                    x_sbs[bi][:],
                    x_ds[bi].rearrange("p (v kb c) -> p v kb c", v=2, kb=KD))

            w1_re = w1_d.rearrange("(fm p) (kb v f) -> p fm kb v f",
                                   p=128, kb=KD, v=2)
            w2_re = w2_d.rearrange("(dn p) (fb v d) -> p dn fb v d",
                                   p=128, fb=KF, v=2)
            dma_x(0)
            for fm in range(KF):
                nc.sync.dma_start(w1_sb[:, fm], w1_re[:, fm])
            if nb > 1:
                dma_x(1)
            for dn in range(KD):
                nc.sync.dma_start(w2_sb[:, dn], w2_re[:, dn])
            for bi in range(2, nb):
                dma_x(bi)
            nc.sync.dma_start(cw_sb[:], cw_d[:])

            def mm1(bi):
                """z = 3-term x@w1 ; h8 = silu fp8 ; hf = silu f32 (per fm)."""
                b0, bs = blocks[bi]
                hfs = []
                x_sb = x_sbs[bi]
                for fm in range(KF):
                    ps = pshpool.tile([128, bs], f32, tag="psh",
                                      name=f"psh{bi}_{fm}")
                    n_i = KD + KD // 2
                    i = 0
                    for kb in range(KD):  # P1: (x_hi,x_hi)x(w_hi,w_lo)
                        nc.tensor.matmul(
                            ps[:],
                            lhsT=w1_sb[:, fm, kb],
                            rhs=x_sb[:, 0, kb].unsqueeze(1)
                                .broadcast_to([128, 2, bs]),
                            start=(i == 0), stop=(i == n_i - 1), perf_mode=DR)
                        i += 1
                    for kb2 in range(0, KD, 2):  # P2: (x_lo,x_lo)x(w_hi,w_hi)
                        nc.tensor.matmul(
                            ps[:],
                            lhsT=w1_sb[:, fm, kb2:kb2 + 2, 0],
                            rhs=x_sb[:, 1, kb2:kb2 + 2],
                            start=(i == 0), stop=(i == n_i - 1), perf_mode=DR)
                        i += 1
                    nc.scalar.activation(h_sb[:, fm, 0, b0:b0 + bs], ps[:],
                                         silu, scale=1.0 / (WS * XS))
                    hf = hfpool.tile([128, bs], f32, tag="hf",
                                     name=f"hf{bi}_{fm}")
                    nc.scalar.activation(hf[:], ps[:], silu,
                                         scale=1.0 / (WS * XS))
                    hfs.append(hf)
                return hfs

            def h_lo(bi, hfs):
                b0, bs = blocks[bi]
                for fm in range(KF):
                    nc.vector.scalar_tensor_tensor(
                        h_sb[:, fm, 1, b0:b0 + bs], hfs[fm][:], 1.0,
                        h_sb[:, fm, 0, b0:b0 + bs],
                        op0=mult, op1=subtract)

            def mm2(bi):
                """y = (3-term h@w2) * cw / WS ; one SP (hw-DGE) DMA/block."""
                b0, bs = blocks[bi]
                ys = ypool.tile([128, KD, bs], bf16, tag="y", name=f"y{bi}")
                for dn in range(KD):
                    ps = psypool.tile([128, bs], f32, tag="psy",
                                      name=f"psy{bi}_{dn}")
                    n_i = KF + KF // 2
                    i = 0
                    for fb in range(KF):  # P1': (h8,h_lo)x(w2_hi,w2_hi)
                        nc.tensor.matmul(
                            ps[:],
                            lhsT=w2_sb[:, dn, fb, 0].unsqueeze(1)
                                .broadcast_to([128, 2, 128]),
                            rhs=h_sb[:, fb, :, b0:b0 + bs],
                            start=(i == 0), stop=(i == n_i - 1), perf_mode=DR)
                        i += 1
                    for fb2 in range(0, KF, 2):  # P2': (h8,h8)x(w2_lo,w2_lo)
                        nc.tensor.matmul(
                            ps[:],
                            lhsT=w2_sb[:, dn, fb2:fb2 + 2, 1],
                            rhs=h_sb[:, fb2:fb2 + 2, 0, b0:b0 + bs],
                            start=(i == 0), stop=(i == n_i - 1), perf_mode=DR)
                        i += 1
                    nc.vector.scalar_tensor_tensor(
                        ys[:, dn], ps[:], 1.0 / WS, cw_sb[:, b0:b0 + bs],
                        op0=mult, op1=mult)
                nc.sync.dma_start(y_re[:, :, b0:b0 + bs], ys[:])

            # ---- software-pipelined emission: mm1 one block ahead of mm2
            hfs = mm1(0)
            h_lo(0, hfs)
            for bi in range(1, nb):
                hfs = mm1(bi)
                mm2(bi - 1)
                h_lo(bi, hfs)
            mm2(nb - 1)

    nc.compile()
    return nc


def _route(x: np.ndarray, gate_w: np.ndarray):
    """Router on host CPU with the reference's exact jax ops/dtypes."""
    try:
        import jax
        import jax.numpy as jnp
        with jax.default_device(jax.devices("cpu")[0]):
            logits = jnp.einsum('bsd,de->bse', jnp.asarray(x),
                                jnp.asarray(gate_w))
            top_logits, top_idx = jax.lax.top_k(logits, TOP_K)
            top_w = jax.nn.softmax(top_logits, axis=-1)
            ti = np.asarray(top_idx).reshape(T, TOP_K)
            tw = np.asarray(top_w).reshape(T, TOP_K).astype(np.float32)
    except Exception:
        # numpy fallback (same selection semantics as jax.lax.top_k)
        logits = (x.reshape(T, D_MODEL) @ gate_w).astype(np.float32)
        i0 = np.argmax(logits, axis=1)
        masked = logits.copy()
        masked[np.arange(T), i0] = -np.inf
        i1 = np.argmax(masked, axis=1)
        v0 = logits[np.arange(T), i0]
        v1 = logits[np.arange(T), i1]
        e1 = np.exp(v1 - v0)
        w0 = 1.0 / (1.0 + e1)
        ti = np.stack([i0, i1], 1)
        tw = np.stack([w0, 1.0 - w0], 1).astype(np.float32)
    return ti, tw


def _hi_lo(a: np.ndarray, F8):
    hi = a.astype(F8)
    lo = (a - hi.astype(np.float32)).astype(F8)
    return hi, lo


def kernel(x: np.ndarray, gate_w: np.ndarray, w1: np.ndarray,
           w2: np.ndarray) -> np.ndarray:
    from concourse.bass_utils import run_bass_kernel_spmd
    import ml_dtypes

    F8 = ml_dtypes.float8_e4m3

    x = np.asarray(x, dtype=np.float32)
    gate_w = np.asarray(gate_w, dtype=np.float32)
    w1 = np.asarray(w1, dtype=np.float32)
    w2 = np.asarray(w2, dtype=np.float32)

    ti, tw = _route(x, gate_w)

    x2d = x.reshape(T, D_MODEL)
    tokens, weights = [], []
    for e in range(N_EXPERTS):
        rows, ks = np.nonzero(ti == e)
        tokens.append(rows)
        weights.append(tw[rows, ks])
    counts = [len(t) for t in tokens]
    C = _round_up(max(max(counts), 512), 32)

    if C not in _PROGRAM_CACHE:
        _PROGRAM_CACHE[C] = _build_program(C)
    nc = _PROGRAM_CACHE[C]

    blocks = _blocks(C)
    in_maps = []
    for e in range(N_EXPERTS):
        n = counts[e]
        # x: [D, C] scaled by XS, hi/lo split, one tensor per token block
        # with row p = [v2, kb8, bs] (partition-contiguous chunks)
        xt = np.zeros((D_MODEL, C), dtype=np.float32)
        if n:
            xt[:, :n] = x2d[tokens[e]].T * XS
        x_hi, x_lo = _hi_lo(xt, F8)
        xs = np.stack([x_hi.reshape(KD, 128, C),
                       x_lo.reshape(KD, 128, C)])      # [v, kb, p, C]
        im = {}
        for bi, (b0, bs) in enumerate(blocks):
            im[f"xt{bi}"] = np.ascontiguousarray(
                xs[:, :, :, b0:b0 + bs].transpose(2, 0, 1, 3)
                ).reshape(128, 2 * KD * bs)

        # w1: row fm*128+p = [kb, v, f128]
        w1_hi, w1_lo = _hi_lo(w1[e] * WS, F8)          # [D, DFF]
        w1v = np.stack([w1_hi, w1_lo]).reshape(2, KD, 128, KF, 128)
        im["w1"] = np.ascontiguousarray(
            w1v.transpose(3, 2, 1, 0, 4)).reshape(KF * 128, KD * 2 * 128)

        # w2: row dn*128+p = [fb, v, d128]
        w2_hi, w2_lo = _hi_lo(w2[e] * WS, F8)          # [DFF, D]
        w2v = np.stack([w2_hi, w2_lo]).reshape(2, KF, 128, KD, 128)
        im["w2"] = np.ascontiguousarray(
            w2v.transpose(3, 2, 1, 0, 4)).reshape(KD * 128, KF * 2 * 128)

        cw = np.zeros((C,), dtype=np.float32)
        cw[:n] = weights[e]
        im["cw"] = np.broadcast_to(cw[None, :], (128, C)).copy()
        in_maps.append(im)

    res = run_bass_kernel_spmd(nc, in_maps, core_ids=list(range(N_CORES)))

    out2d = np.zeros((T, D_MODEL), dtype=np.float32)
    for e in range(N_EXPERTS):
        n = counts[e]
        if n:
            out2d[tokens[e]] += res.results[e]["y"].astype(np.float32).T[:n]

    LAST_BUILD["nc"] = nc
    LAST_BUILD["C"] = C
    return out2d.reshape(B, S, D_MODEL)


# revision 20
# speedup vs baseline: 1.3884x; 1.0137x over previous
"""MoE (top-2 of 8 experts) Trainium2 kernel — fp8 DoubleRow version.

Expert-parallel across the 8 NeuronCores. The (cheap) router runs on host
CPU; each core runs one expert's MLP over its routed tokens.

Device math uses fp8(e4m3) matmuls in DoubleRow perf mode (K=256 per
instruction at 0.5 cycles/row — 2x bf16 PE throughput) with a 3-term
residual-compensation scheme to stay well inside the accuracy budget:

    x @ w  ~=  x_hi@w_hi + x_hi@w_lo + x_lo@w_hi

where *_hi = fp8(v) and *_lo = fp8(v - *_hi). Weights are pre-scaled by
2^5 (and x by 2^2) on the host so fp8 subnormals are avoided; the scales
are folded into the silu input scale and the combine weights. The hidden
activation h is split on device: h8 = silu(psum) in fp8 (ACT), h_lo =
hf - h8 (DVE), both feeding matmul 2. Both matmuls keep tokens on the
moving/free dim, so PE cost is exactly 192*C cycles per core.

Self-contained: only environment packages (numpy/jax/concourse/ml_dtypes).
"""

import os
import sys

import numpy as np

# concourse ships on sys.path via the container's sitecustomize
# (/root/.axon_site/_ro/trn_rl_repo); /opt copy is a fallback only.
if "/opt/trn_rl_repo" not in sys.path:
    sys.path.append("/opt/trn_rl_repo")

B, S, D_MODEL, D_FF, N_EXPERTS, TOP_K = 2, 2048, 1024, 2048, 8, 2
T = B * S
N_CORES = 8
KD = D_MODEL // 128   # 8
KF = D_FF // 128      # 16
WS = 32.0             # weight pre-scale (2^5)
XS = 4.0              # x pre-scale (2^2)
NWARM = int(os.environ.get("BASS_MOE_NWARM", "24"))

_PROGRAM_CACHE: dict = {}
LAST_BUILD = {}


def _round_up(v: int, m: int) -> int:
    return ((v + m - 1) // m) * m


def _blocks(C: int):
    """Token blocks: 320 first (so the x(b0) DMA lands before the first w1
    chunk), then 512s; the remainder lands in the last (small) block so the
    post-PE tail is short."""
    out = []
    b0 = 0
    while b0 < C:
        bs = min(384 if b0 == 0 and C > 512 else 512, C - b0)
        out.append((b0, bs))
        b0 += bs
    return out


def _build_program(C: int):
    import concourse.tile as tile
    from concourse import bacc, mybir

    f8 = mybir.dt.float8e4
    f32 = mybir.dt.float32
    bf16 = mybir.dt.bfloat16
    DR = mybir.MatmulPerfMode.DoubleRow
    silu = mybir.ActivationFunctionType.Silu
    mult = mybir.AluOpType.mult
    subtract = mybir.AluOpType.subtract

    nc = bacc.Bacc("TRN2", target_bir_lowering=False, debug=False,
                   num_devices=N_CORES)
    blocks = _blocks(C)
    nb = len(blocks)

    # All inputs are packed on the host so every DMA chunk is
    # partition-contiguous (>=512B descriptor runs -> full DMA rate) at
    # exactly the granularity the PE consumes:
    #   w1: row fm*128+p holds [kb8, v2, f128]  -> 16 per-fm chunks
    #   w2: row dn*128+p holds [fb16, v2, d128] ->  8 per-dn chunks
    #   x : one tensor per block, row p holds [v2, kb8, bs]
    w1_d = nc.dram_tensor("w1", [KF * 128, KD * 2 * 128], f8,
                          kind="ExternalInput").ap()
    w2_d = nc.dram_tensor("w2", [KD * 128, KF * 2 * 128], f8,
                          kind="ExternalInput").ap()
    x_ds = [nc.dram_tensor(f"xt{bi}", [128, 2 * KD * bs], f8,
                           kind="ExternalInput").ap()
            for bi, (b0, bs) in enumerate(blocks)]
    cw_d = nc.dram_tensor("cw", [128, C], f32, kind="ExternalInput").ap()
    y_d = nc.dram_tensor("y", [D_MODEL, C], bf16, kind="ExternalOutput").ap()

    y_re = y_d.rearrange("(dn p) c -> p dn c", p=128)

    with tile.TileContext(nc) as tc:
        with (
            tc.tile_pool(name="big", bufs=1) as big,
            tc.tile_pool(name="hfpool", bufs=3) as hfpool,
            tc.tile_pool(name="ypool", bufs=3) as ypool,
            tc.tile_pool(name="psh", bufs=3, space="PSUM") as pshpool,
            tc.tile_pool(name="psy", bufs=5, space="PSUM") as psypool,
        ):
            x_sbs = [big.tile([128, 2, KD, bs], f8, name=f"x_sb{bi}")
                     for bi, (b0, bs) in enumerate(blocks)]
            w1_sb = big.tile([128, KF, KD, 2, 128], f8, name="w1_sb")
            w2_sb = big.tile([128, KD, KF, 2, 128], f8, name="w2_sb")
            h_sb = big.tile([128, KF, 2, C], f8, name="h_sb")
            cw_sb = big.tile([128, C], f32, name="cw_sb")
            warm = big.tile([128, 2, 256], f8, name="warm")

            # PE warmup: ramp the p-state while input DMAs stream. Garbage
            # SBUF bytes can decode as fp8 NaN/Inf and wedge the exec unit,
            # so zero the tile first (Pool dispatches fastest).
            nc.vector.memset(warm[:], 0)
            for i in range(NWARM):
                wps = pshpool.tile([128, 128], f32, tag="psh", name=f"wps{i}")
                nc.tensor.matmul(wps[:], lhsT=warm[:, :, 0:128],
                                 rhs=warm[:, :, 0:128],
                                 start=True, stop=True, perf_mode=DR)

            # ---- input DMAs, in consumption order
            def dma_x(bi):
                b0, bs = blocks[bi]
                nc.sync.dma_start(
                    x_sbs[bi][:],
                    x_ds[bi].rearrange("p (v kb c) -> p v kb c", v=2, kb=KD))

            w1_re = w1_d.rearrange("(fm p) (kb v f) -> p fm kb v f",
                                   p=128, kb=KD, v=2)
            w2_re = w2_d.rearrange("(dn p) (fb v d) -> p dn fb v d",
                                   p=128, fb=KF, v=2)
            nc.sync.dma_start(w1_sb[:, 0], w1_re[:, 0])
            dma_x(0)
            for fm in range(1, KF):
                nc.sync.dma_start(w1_sb[:, fm], w1_re[:, fm])
            if nb > 1:
                dma_x(1)
            for dn in range(KD):
                nc.sync.dma_start(w2_sb[:, dn], w2_re[:, dn])
            for bi in range(2, nb):
                dma_x(bi)
            nc.sync.dma_start(cw_sb[:], cw_d[:])

            def mm1(bi):
                """z = 3-term x@w1 ; h8 = silu fp8 ; hf = silu f32 (per fm)."""
                b0, bs = blocks[bi]
                hfs = []
                x_sb = x_sbs[bi]
                for fm in range(KF):
                    ps = pshpool.tile([128, bs], f32, tag="psh",
                                      name=f"psh{bi}_{fm}")
                    n_i = KD + KD // 2
                    i = 0
                    for kb in range(KD):  # P1: (x_hi,x_hi)x(w_hi,w_lo)
                        nc.tensor.matmul(
                            ps[:],
                            lhsT=w1_sb[:, fm, kb],
                            rhs=x_sb[:, 0, kb].unsqueeze(1)
                                .broadcast_to([128, 2, bs]),
                            start=(i == 0), stop=(i == n_i - 1), perf_mode=DR)
                        i += 1
                    for kb2 in range(0, KD, 2):  # P2: (x_lo,x_lo)x(w_hi,w_hi)
                        nc.tensor.matmul(
                            ps[:],
                            lhsT=w1_sb[:, fm, kb2:kb2 + 2, 0],
                            rhs=x_sb[:, 1, kb2:kb2 + 2],
                            start=(i == 0), stop=(i == n_i - 1), perf_mode=DR)
                        i += 1
                    nc.scalar.activation(h_sb[:, fm, 0, b0:b0 + bs], ps[:],
                                         silu, scale=1.0 / (WS * XS))
                    hf = hfpool.tile([128, bs], f32, tag="hf",
                                     name=f"hf{bi}_{fm}")
                    nc.scalar.activation(hf[:], ps[:], silu,
                                         scale=1.0 / (WS * XS))
                    hfs.append(hf)
                return hfs

            def h_lo(bi, hfs):
                b0, bs = blocks[bi]
                for fm in range(KF):
                    nc.vector.scalar_tensor_tensor(
                        h_sb[:, fm, 1, b0:b0 + bs], hfs[fm][:], 1.0,
                        h_sb[:, fm, 0, b0:b0 + bs],
                        op0=mult, op1=subtract)

            def mm2(bi):
                """y = (3-term h@w2) * cw / WS ; per-dn SP (hw-DGE) DMAs."""
                b0, bs = blocks[bi]
                for dn in range(KD):
                    ps = psypool.tile([128, bs], f32, tag="psy",
                                      name=f"psy{bi}_{dn}")
                    n_i = KF + KF // 2
                    i = 0
                    for fb in range(KF):  # P1': (h8,h_lo)x(w2_hi,w2_hi)
                        nc.tensor.matmul(
                            ps[:],
                            lhsT=w2_sb[:, dn, fb, 0].unsqueeze(1)
                                .broadcast_to([128, 2, 128]),
                            rhs=h_sb[:, fb, :, b0:b0 + bs],
                            start=(i == 0), stop=(i == n_i - 1), perf_mode=DR)
                        i += 1
                    for fb2 in range(0, KF, 2):  # P2': (h8,h8)x(w2_lo,w2_lo)
                        nc.tensor.matmul(
                            ps[:],
                            lhsT=w2_sb[:, dn, fb2:fb2 + 2, 1],
                            rhs=h_sb[:, fb2:fb2 + 2, 0, b0:b0 + bs],
                            start=(i == 0), stop=(i == n_i - 1), perf_mode=DR)
                        i += 1
                    ys = ypool.tile([128, bs], bf16, tag="y",
                                    name=f"y{bi}_{dn}")
                    nc.vector.scalar_tensor_tensor(
                        ys[:], ps[:], 1.0 / WS, cw_sb[:, b0:b0 + bs],
                        op0=mult, op1=mult)
                    nc.sync.dma_start(y_re[:, dn, b0:b0 + bs], ys[:])

            # ---- software-pipelined emission: mm1 one block ahead of mm2
            hfs = mm1(0)
            h_lo(0, hfs)
            for bi in range(1, nb):
                hfs = mm1(bi)
                mm2(bi - 1)
                h_lo(bi, hfs)
            mm2(nb - 1)

    nc.compile()
    return nc


def _route(x: np.ndarray, gate_w: np.ndarray):
    """Router on host CPU with the reference's exact jax ops/dtypes."""
    try:
        import jax
        import jax.numpy as jnp
        with jax.default_device(jax.devices("cpu")[0]):
            logits = jnp.einsum('bsd,de->bse', jnp.asarray(x),
                                jnp.asarray(gate_w))
            top_logits, top_idx = jax.lax.top_k(logits, TOP_K)
            top_w = jax.nn.softmax(top_logits, axis=-1)
            ti = np.asarray(top_idx).reshape(T, TOP_K)
            tw = np.asarray(top_w).reshape(T, TOP_K).astype(np.float32)
    except Exception:
        # numpy fallback (same selection semantics as jax.lax.top_k)
        logits = (x.reshape(T, D_MODEL) @ gate_w).astype(np.float32)
        i0 = np.argmax(logits, axis=1)
        masked = logits.copy()
        masked[np.arange(T), i0] = -np.inf
        i1 = np.argmax(masked, axis=1)
        v0 = logits[np.arange(T), i0]
        v1 = logits[np.arange(T), i1]
        e1 = np.exp(v1 - v0)
        w0 = 1.0 / (1.0 + e1)
        ti = np.stack([i0, i1], 1)
        tw = np.stack([w0, 1.0 - w0], 1).astype(np.float32)
    return ti, tw


def _hi_lo(a: np.ndarray, F8):
    hi = a.astype(F8)
    lo = (a - hi.astype(np.float32)).astype(F8)
    return hi, lo


def kernel(x: np.ndarray, gate_w: np.ndarray, w1: np.ndarray,
           w2: np.ndarray) -> np.ndarray:
    from concourse.bass_utils import run_bass_kernel_spmd
    import ml_dtypes

    F8 = ml_dtypes.float8_e4m3

    x = np.asarray(x, dtype=np.float32)
    gate_w = np.asarray(gate_w, dtype=np.float32)
    w1 = np.asarray(w1, dtype=np.float32)
    w2 = np.asarray(w2, dtype=np.float32)

    ti, tw = _route(x, gate_w)

    x2d = x.reshape(T, D_MODEL)
    tokens, weights = [], []
    for e in range(N_EXPERTS):
        rows, ks = np.nonzero(ti == e)
        tokens.append(rows)
        weights.append(tw[rows, ks])
    counts = [len(t) for t in tokens]
    C = _round_up(max(max(counts), 512), 32)

    if C not in _PROGRAM_CACHE:
        _PROGRAM_CACHE[C] = _build_program(C)
    nc = _PROGRAM_CACHE[C]

    blocks = _blocks(C)
    in_maps = []
    for e in range(N_EXPERTS):
        n = counts[e]
        # x: [D, C] scaled by XS, hi/lo split, one tensor per token block
        # with row p = [v2, kb8, bs] (partition-contiguous chunks)
        xt = np.zeros((D_MODEL, C), dtype=np.float32)
        if n:
            xt[:, :n] = x2d[tokens[e]].T * XS
        x_hi, x_lo = _hi_lo(xt, F8)
        xs = np.stack([x_hi.reshape(KD, 128, C),
                       x_lo.reshape(KD, 128, C)])      # [v, kb, p, C]
        im = {}
        for bi, (b0, bs) in enumerate(blocks):
            im[f"xt{bi}"] = np.ascontiguousarray(
                xs[:, :, :, b0:b0 + bs].transpose(2, 0, 1, 3)
                ).reshape(128, 2 * KD * bs)

        # w1: row fm*128+p = [kb, v, f128]
        w1_hi, w1_lo = _hi_lo(w1[e] * WS, F8)          # [D, DFF]
        w1v = np.stack([w1_hi, w1_lo]).reshape(2, KD, 128, KF, 128)
        im["w1"] = np.ascontiguousarray(
            w1v.transpose(3, 2, 1, 0, 4)).reshape(KF * 128, KD * 2 * 128)

        # w2: row dn*128+p = [fb, v, d128]
        w2_hi, w2_lo = _hi_lo(w2[e] * WS, F8)          # [DFF, D]
        w2v = np.stack([w2_hi, w2_lo]).reshape(2, KF, 128, KD, 128)
        im["w2"] = np.ascontiguousarray(
            w2v.transpose(3, 2, 1, 0, 4)).reshape(KD * 128, KF * 2 * 128)

        cw = np.zeros((C,), dtype=np.float32)
        cw[:n] = weights[e]
        im["cw"] = np.broadcast_to(cw[None, :], (128, C)).copy()
        in_maps.append(im)

    res = run_bass_kernel_spmd(nc, in_maps, core_ids=list(range(N_CORES)))

    out2d = np.zeros((T, D_MODEL), dtype=np.float32)
    for e in range(N_EXPERTS):
        n = counts[e]
        if n:
            out2d[tokens[e]] += res.results[e]["y"].astype(np.float32).T[:n]

    LAST_BUILD["nc"] = nc
    LAST_BUILD["C"] = C
    return out2d.reshape(B, S, D_MODEL)


# revision 21
# speedup vs baseline: 1.4466x; 1.0419x over previous
"""MoE (top-2 of 8 experts) Trainium2 kernel — fp8 DoubleRow version.

Expert-parallel across the 8 NeuronCores. The (cheap) router runs on host
CPU; each core runs one expert's MLP over its routed tokens.

Device math uses fp8(e4m3) matmuls in DoubleRow perf mode (K=256 per
instruction at 0.5 cycles/row — 2x bf16 PE throughput) with a 3-term
residual-compensation scheme to stay well inside the accuracy budget:

    x @ w  ~=  x_hi@w_hi + x_hi@w_lo + x_lo@w_hi

where *_hi = fp8(v) and *_lo = fp8(v - *_hi). Weights are pre-scaled by
2^5 (and x by 2^2) on the host so fp8 subnormals are avoided; the scales
are folded into the silu input scale and the combine weights. The hidden
activation h is split on device: h8 = silu(psum) in fp8 (ACT), h_lo =
hf - h8 (DVE), both feeding matmul 2. Both matmuls keep tokens on the
moving/free dim, so PE cost is exactly 192*C cycles per core.

Self-contained: only environment packages (numpy/jax/concourse/ml_dtypes).
"""

import os
import sys

import numpy as np

# concourse ships on sys.path via the container's sitecustomize
# (/root/.axon_site/_ro/trn_rl_repo); /opt copy is a fallback only.
if "/opt/trn_rl_repo" not in sys.path:
    sys.path.append("/opt/trn_rl_repo")

B, S, D_MODEL, D_FF, N_EXPERTS, TOP_K = 2, 2048, 1024, 2048, 8, 2
T = B * S
N_CORES = 8
KD = D_MODEL // 128   # 8
KF = D_FF // 128      # 16
WS = 32.0             # weight pre-scale (2^5)
XS = 4.0              # x pre-scale (2^2)
NWARM = int(os.environ.get("BASS_MOE_NWARM", "24"))

_PROGRAM_CACHE: dict = {}
LAST_BUILD = {}


def _round_up(v: int, m: int) -> int:
    return ((v + m - 1) // m) * m


def _blocks(C: int):
    """Token blocks: 320 first (so the x(b0) DMA lands before the first w1
    chunk), then 512s; the remainder lands in the last (small) block so the
    post-PE tail is short."""
    out = []
    b0 = 0
    while b0 < C:
        bs = min(384 if b0 == 0 and C > 512 else 512, C - b0)
        out.append((b0, bs))
        b0 += bs
    return out


def _build_program(C: int):
    import concourse.tile as tile
    from concourse import bacc, mybir

    f8 = mybir.dt.float8e4
    f32 = mybir.dt.float32
    bf16 = mybir.dt.bfloat16
    DR = mybir.MatmulPerfMode.DoubleRow
    silu = mybir.ActivationFunctionType.Silu
    mult = mybir.AluOpType.mult
    subtract = mybir.AluOpType.subtract

    nc = bacc.Bacc("TRN2", target_bir_lowering=False, debug=False,
                   num_devices=N_CORES)
    blocks = _blocks(C)
    nb = len(blocks)

    # All inputs are packed on the host so every DMA chunk is
    # partition-contiguous (>=512B descriptor runs -> full DMA rate) at
    # exactly the granularity the PE consumes:
    #   w1: row fm*128+p holds [kb8, v2, f128]  -> 16 per-fm chunks
    #   w2: row dn*128+p holds [fb16, v2, d128] ->  8 per-dn chunks
    #   x : one tensor per block, row p holds [v2, kb8, bs]
    w1_d = nc.dram_tensor("w1", [KF * 128, KD * 2 * 128], f8,
                          kind="ExternalInput").ap()
    w2_d = nc.dram_tensor("w2", [KD * 128, KF * 2 * 128], f8,
                          kind="ExternalInput").ap()
    x_ds = [nc.dram_tensor(f"xt{bi}", [128, 2 * KD * bs], f8,
                           kind="ExternalInput").ap()
            for bi, (b0, bs) in enumerate(blocks)]
    cw_d = nc.dram_tensor("cw", [128, C], f32, kind="ExternalInput").ap()
    y_d = nc.dram_tensor("y", [D_MODEL, C], bf16, kind="ExternalOutput").ap()

    y_re = y_d.rearrange("(dn p) c -> p dn c", p=128)

    with tile.TileContext(nc) as tc:
        with (
            tc.tile_pool(name="big", bufs=1) as big,
            tc.tile_pool(name="hfpool", bufs=3) as hfpool,
            tc.tile_pool(name="ypool", bufs=3) as ypool,
            tc.tile_pool(name="psh", bufs=4, space="PSUM") as pshpool,
            tc.tile_pool(name="psy", bufs=3, space="PSUM") as psypool,
        ):
            x_sbs = [big.tile([128, 2, KD, bs], f8, name=f"x_sb{bi}")
                     for bi, (b0, bs) in enumerate(blocks)]
            w1_sb = big.tile([128, KF, KD, 2, 128], f8, name="w1_sb")
            w2_sb = big.tile([128, KD, KF, 2, 128], f8, name="w2_sb")
            h_sb = big.tile([128, KF, 2, C], f8, name="h_sb")
            cw_sb = big.tile([128, C], f32, name="cw_sb")
            warm = big.tile([128, 2, 256], f8, name="warm")

            # PE warmup: ramp the p-state while input DMAs stream. Garbage
            # SBUF bytes can decode as fp8 NaN/Inf and wedge the exec unit,
            # so zero the tile first (Pool dispatches fastest).
            nc.vector.memset(warm[:], 0)
            for i in range(NWARM):
                wps = pshpool.tile([128, 128], f32, tag="psh", name=f"wps{i}")
                nc.tensor.matmul(wps[:], lhsT=warm[:, :, 0:128],
                                 rhs=warm[:, :, 0:128],
                                 start=True, stop=True, perf_mode=DR)

            # ---- input DMAs, in consumption order
            def dma_x(bi):
                b0, bs = blocks[bi]
                nc.sync.dma_start(
                    x_sbs[bi][:],
                    x_ds[bi].rearrange("p (v kb c) -> p v kb c", v=2, kb=KD))

            w1_re = w1_d.rearrange("(fm p) (kb v f) -> p fm kb v f",
                                   p=128, kb=KD, v=2)
            w2_re = w2_d.rearrange("(dn p) (fb v d) -> p dn fb v d",
                                   p=128, fb=KF, v=2)
            nc.sync.dma_start(w1_sb[:, 0], w1_re[:, 0])
            dma_x(0)
            for fm in range(1, KF):
                nc.sync.dma_start(w1_sb[:, fm], w1_re[:, fm])
            if nb > 1:
                dma_x(1)
            for dn in range(KD):
                nc.sync.dma_start(w2_sb[:, dn], w2_re[:, dn])
            for bi in range(2, nb):
                dma_x(bi)
            nc.sync.dma_start(cw_sb[:], cw_d[:])

            def mm1(bi):
                """z = 3-term x@w1 ; h8 = silu fp8 ; hf = silu f32 (per fm)."""
                b0, bs = blocks[bi]
                hfs = []
                x_sb = x_sbs[bi]
                for fm in range(KF):
                    ps = pshpool.tile([128, bs], f32, tag="psh",
                                      name=f"psh{bi}_{fm}")
                    n_i = KD + KD // 2
                    i = 0
                    for kb in range(KD):  # P1: (x_hi,x_hi)x(w_hi,w_lo)
                        nc.tensor.matmul(
                            ps[:],
                            lhsT=w1_sb[:, fm, kb],
                            rhs=x_sb[:, 0, kb].unsqueeze(1)
                                .broadcast_to([128, 2, bs]),
                            start=(i == 0), stop=(i == n_i - 1), perf_mode=DR)
                        i += 1
                    for kb2 in range(0, KD, 2):  # P2: (x_lo,x_lo)x(w_hi,w_hi)
                        nc.tensor.matmul(
                            ps[:],
                            lhsT=w1_sb[:, fm, kb2:kb2 + 2, 0],
                            rhs=x_sb[:, 1, kb2:kb2 + 2],
                            start=(i == 0), stop=(i == n_i - 1), perf_mode=DR)
                        i += 1
                    nc.scalar.activation(h_sb[:, fm, 0, b0:b0 + bs], ps[:],
                                         silu, scale=1.0 / (WS * XS))
                    hf = hfpool.tile([128, bs], f32, tag="hf",
                                     name=f"hf{bi}_{fm}")
                    nc.scalar.activation(hf[:], ps[:], silu,
                                         scale=1.0 / (WS * XS))
                    hfs.append(hf)
                return hfs

            def h_lo(bi, hfs):
                b0, bs = blocks[bi]
                for fm in range(KF):
                    nc.vector.scalar_tensor_tensor(
                        h_sb[:, fm, 1, b0:b0 + bs], hfs[fm][:], 1.0,
                        h_sb[:, fm, 0, b0:b0 + bs],
                        op0=mult, op1=subtract)

            def mm2(bi):
                """y = (3-term h@w2) * cw / WS ; per-dn SP (hw-DGE) DMAs."""
                b0, bs = blocks[bi]
                for dn in range(KD):
                    ps = psypool.tile([128, bs], f32, tag="psy",
                                      name=f"psy{bi}_{dn}")
                    n_i = KF + KF // 2
                    i = 0
                    for fb in range(KF):  # P1': (h8,h_lo)x(w2_hi,w2_hi)
                        nc.tensor.matmul(
                            ps[:],
                            lhsT=w2_sb[:, dn, fb, 0].unsqueeze(1)
                                .broadcast_to([128, 2, 128]),
                            rhs=h_sb[:, fb, :, b0:b0 + bs],
                            start=(i == 0), stop=(i == n_i - 1), perf_mode=DR)
                        i += 1
                    for fb2 in range(0, KF, 2):  # P2': (h8,h8)x(w2_lo,w2_lo)
                        nc.tensor.matmul(
                            ps[:],
                            lhsT=w2_sb[:, dn, fb2:fb2 + 2, 1],
                            rhs=h_sb[:, fb2:fb2 + 2, 0, b0:b0 + bs],
                            start=(i == 0), stop=(i == n_i - 1), perf_mode=DR)
                        i += 1
                    ys = ypool.tile([128, bs], bf16, tag="y",
                                    name=f"y{bi}_{dn}")
                    nc.vector.scalar_tensor_tensor(
                        ys[:], ps[:], 1.0 / WS, cw_sb[:, b0:b0 + bs],
                        op0=mult, op1=mult)
                    nc.sync.dma_start(y_re[:, dn, b0:b0 + bs], ys[:])

            # ---- software-pipelined emission: mm1 one block ahead of mm2
            hfs = mm1(0)
            h_lo(0, hfs)
            for bi in range(1, nb):
                hfs = mm1(bi)
                mm2(bi - 1)
                h_lo(bi, hfs)
            mm2(nb - 1)

    nc.compile()
    return nc


def _route(x: np.ndarray, gate_w: np.ndarray):
    """Router on host CPU with the reference's exact jax ops/dtypes."""
    try:
        import jax
        import jax.numpy as jnp
        with jax.default_device(jax.devices("cpu")[0]):
            logits = jnp.einsum('bsd,de->bse', jnp.asarray(x),
                                jnp.asarray(gate_w))
            top_logits, top_idx = jax.lax.top_k(logits, TOP_K)
            top_w = jax.nn.softmax(top_logits, axis=-1)
            ti = np.asarray(top_idx).reshape(T, TOP_K)
            tw = np.asarray(top_w).reshape(T, TOP_K).astype(np.float32)
    except Exception:
        # numpy fallback (same selection semantics as jax.lax.top_k)
        logits = (x.reshape(T, D_MODEL) @ gate_w).astype(np.float32)
        i0 = np.argmax(logits, axis=1)
        masked = logits.copy()
        masked[np.arange(T), i0] = -np.inf
        i1 = np.argmax(masked, axis=1)
        v0 = logits[np.arange(T), i0]
        v1 = logits[np.arange(T), i1]
        e1 = np.exp(v1 - v0)
        w0 = 1.0 / (1.0 + e1)
        ti = np.stack([i0, i1], 1)
        tw = np.stack([w0, 1.0 - w0], 1).astype(np.float32)
    return ti, tw


def _hi_lo(a: np.ndarray, F8):
    hi = a.astype(F8)
    lo = (a - hi.astype(np.float32)).astype(F8)
    return hi, lo


def kernel(x: np.ndarray, gate_w: np.ndarray, w1: np.ndarray,
           w2: np.ndarray) -> np.ndarray:
    from concourse.bass_utils import run_bass_kernel_spmd
    import ml_dtypes

    F8 = ml_dtypes.float8_e4m3

    x = np.asarray(x, dtype=np.float32)
    gate_w = np.asarray(gate_w, dtype=np.float32)
    w1 = np.asarray(w1, dtype=np.float32)
    w2 = np.asarray(w2, dtype=np.float32)

    ti, tw = _route(x, gate_w)

    x2d = x.reshape(T, D_MODEL)
    tokens, weights = [], []
    for e in range(N_EXPERTS):
        rows, ks = np.nonzero(ti == e)
        tokens.append(rows)
        weights.append(tw[rows, ks])
    counts = [len(t) for t in tokens]
    C = _round_up(max(max(counts), 512), 32)

    if C not in _PROGRAM_CACHE:
        _PROGRAM_CACHE[C] = _build_program(C)
    nc = _PROGRAM_CACHE[C]

    blocks = _blocks(C)
    in_maps = []
    for e in range(N_EXPERTS):
        n = counts[e]
        # x: [D, C] scaled by XS, hi/lo split, one tensor per token block
        # with row p = [v2, kb8, bs] (partition-contiguous chunks)
        xt = np.zeros((D_MODEL, C), dtype=np.float32)
        if n:
            xt[:, :n] = x2d[tokens[e]].T * XS
        x_hi, x_lo = _hi_lo(xt, F8)
        xs = np.stack([x_hi.reshape(KD, 128, C),
                       x_lo.reshape(KD, 128, C)])      # [v, kb, p, C]
        im = {}
        for bi, (b0, bs) in enumerate(blocks):
            im[f"xt{bi}"] = np.ascontiguousarray(
                xs[:, :, :, b0:b0 + bs].transpose(2, 0, 1, 3)
                ).reshape(128, 2 * KD * bs)

        # w1: row fm*128+p = [kb, v, f128]
        w1_hi, w1_lo = _hi_lo(w1[e] * WS, F8)          # [D, DFF]
        w1v = np.stack([w1_hi, w1_lo]).reshape(2, KD, 128, KF, 128)
        im["w1"] = np.ascontiguousarray(
            w1v.transpose(3, 2, 1, 0, 4)).reshape(KF * 128, KD * 2 * 128)

        # w2: row dn*128+p = [fb, v, d128]
        w2_hi, w2_lo = _hi_lo(w2[e] * WS, F8)          # [DFF, D]
        w2v = np.stack([w2_hi, w2_lo]).reshape(2, KF, 128, KD, 128)
        im["w2"] = np.ascontiguousarray(
            w2v.transpose(3, 2, 1, 0, 4)).reshape(KD * 128, KF * 2 * 128)

        cw = np.zeros((C,), dtype=np.float32)
        cw[:n] = weights[e]
        im["cw"] = np.broadcast_to(cw[None, :], (128, C)).copy()
        in_maps.append(im)

    res = run_bass_kernel_spmd(nc, in_maps, core_ids=list(range(N_CORES)))

    out2d = np.zeros((T, D_MODEL), dtype=np.float32)
    for e in range(N_EXPERTS):
        n = counts[e]
        if n:
            out2d[tokens[e]] += res.results[e]["y"].astype(np.float32).T[:n]

    LAST_BUILD["nc"] = nc
    LAST_BUILD["C"] = C
    return out2d.reshape(B, S, D_MODEL)


# revision 22
# speedup vs baseline: 1.4789x; 1.0223x over previous
"""MoE (top-2 of 8 experts) Trainium2 kernel — fp8 DoubleRow version.

Expert-parallel across the 8 NeuronCores. The (cheap) router runs on host
CPU; each core runs one expert's MLP over its routed tokens.

Device math uses fp8(e4m3) matmuls in DoubleRow perf mode (K=256 per
instruction at 0.5 cycles/row — 2x bf16 PE throughput) with a 3-term
residual-compensation scheme to stay well inside the accuracy budget:

    x @ w  ~=  x_hi@w_hi + x_hi@w_lo + x_lo@w_hi

where *_hi = fp8(v) and *_lo = fp8(v - *_hi). Weights are pre-scaled by
2^5 (and x by 2^2) on the host so fp8 subnormals are avoided; the scales
are folded into the silu input scale and the combine weights. The hidden
activation h is split on device: h8 = silu(psum) in fp8 (ACT), h_lo =
hf - h8 (DVE), both feeding matmul 2. Both matmuls keep tokens on the
moving/free dim, so PE cost is exactly 192*C cycles per core.

Self-contained: only environment packages (numpy/jax/concourse/ml_dtypes).
"""

import os
import sys

import numpy as np

# concourse ships on sys.path via the container's sitecustomize
# (/root/.axon_site/_ro/trn_rl_repo); /opt copy is a fallback only.
if "/opt/trn_rl_repo" not in sys.path:
    sys.path.append("/opt/trn_rl_repo")

B, S, D_MODEL, D_FF, N_EXPERTS, TOP_K = 2, 2048, 1024, 2048, 8, 2
T = B * S
N_CORES = 8
KD = D_MODEL // 128   # 8
KF = D_FF // 128      # 16
WS = 32.0             # weight pre-scale (2^5)
XS = 4.0              # x pre-scale (2^2)
NWARM = int(os.environ.get("BASS_MOE_NWARM", "24"))

_PROGRAM_CACHE: dict = {}
LAST_BUILD = {}


def _round_up(v: int, m: int) -> int:
    return ((v + m - 1) // m) * m


def _blocks(C: int):
    """Token blocks: 320 first (so the x(b0) DMA lands before the first w1
    chunk), then 512s; the remainder lands in the last (small) block so the
    post-PE tail is short."""
    out = []
    b0 = 0
    while b0 < C:
        bs = min(384 if b0 == 0 and C > 512 else 512, C - b0)
        out.append((b0, bs))
        b0 += bs
    return out


def _build_program(C: int):
    import concourse.tile as tile
    from concourse import bacc, mybir

    f8 = mybir.dt.float8e4
    f32 = mybir.dt.float32
    bf16 = mybir.dt.bfloat16
    DR = mybir.MatmulPerfMode.DoubleRow
    silu = mybir.ActivationFunctionType.Silu
    mult = mybir.AluOpType.mult
    subtract = mybir.AluOpType.subtract

    nc = bacc.Bacc("TRN2", target_bir_lowering=False, debug=False,
                   num_devices=N_CORES)
    blocks = _blocks(C)
    nb = len(blocks)

    # All inputs are packed on the host so every DMA chunk is
    # partition-contiguous (>=512B descriptor runs -> full DMA rate) at
    # exactly the granularity the PE consumes:
    #   w1: row fm*128+p holds [kb8, v2, f128]  -> 16 per-fm chunks
    #   w2: row dn*128+p holds [fb16, v2, d128] ->  8 per-dn chunks
    #   x : one tensor per block, row p holds [v2, kb8, bs]
    w1_d = nc.dram_tensor("w1", [KF * 128, KD * 2 * 128], f8,
                          kind="ExternalInput").ap()
    w2_d = nc.dram_tensor("w2", [KD * 128, KF * 2 * 128], f8,
                          kind="ExternalInput").ap()
    x_ds = [nc.dram_tensor(f"xt{bi}", [128, 2 * KD * bs], f8,
                           kind="ExternalInput").ap()
            for bi, (b0, bs) in enumerate(blocks)]
    cw_d = nc.dram_tensor("cw", [128, C], f32, kind="ExternalInput").ap()
    y_d = nc.dram_tensor("y", [D_MODEL, C], bf16, kind="ExternalOutput").ap()

    y_re = y_d.rearrange("(dn p) c -> p dn c", p=128)

    with tile.TileContext(nc) as tc:
        with (
            tc.tile_pool(name="big", bufs=1) as big,
            tc.tile_pool(name="hfpool", bufs=3) as hfpool,
            tc.tile_pool(name="ypool", bufs=3) as ypool,
            tc.tile_pool(name="psh", bufs=4, space="PSUM") as pshpool,
            tc.tile_pool(name="psy", bufs=3, space="PSUM") as psypool,
        ):
            x_sbs = [big.tile([128, 2, KD, bs], f8, name=f"x_sb{bi}")
                     for bi, (b0, bs) in enumerate(blocks)]
            w1_sb = big.tile([128, KF, KD, 2, 128], f8, name="w1_sb")
            w2_sb = big.tile([128, KD, KF, 2, 128], f8, name="w2_sb")
            h_sb = big.tile([128, KF, 2, C], f8, name="h_sb")
            cw_sb = big.tile([128, C], f32, name="cw_sb")
            warm = big.tile([128, 2, 256], f8, name="warm")

            # PE warmup: ramp the p-state while input DMAs stream. Garbage
            # SBUF bytes can decode as fp8 NaN/Inf and wedge the exec unit,
            # so zero the tile first (Pool dispatches fastest).
            nc.vector.memset(warm[:], 0)
            for i in range(NWARM):
                wps = pshpool.tile([128, 128], f32, tag="psh", name=f"wps{i}")
                nc.tensor.matmul(wps[:], lhsT=warm[:, :, 0:128],
                                 rhs=warm[:, :, 0:128],
                                 start=True, stop=True, perf_mode=DR)

            # ---- input DMAs, in consumption order
            def dma_x(bi):
                b0, bs = blocks[bi]
                nc.sync.dma_start(
                    x_sbs[bi][:],
                    x_ds[bi].rearrange("p (v kb c) -> p v kb c", v=2, kb=KD))

            w1_re = w1_d.rearrange("(fm p) (kb v f) -> p fm kb v f",
                                   p=128, kb=KD, v=2)
            w2_re = w2_d.rearrange("(dn p) (fb v d) -> p dn fb v d",
                                   p=128, fb=KF, v=2)
            nc.sync.dma_start(w1_sb[:, 0], w1_re[:, 0])
            dma_x(0)
            for fm in range(1, KF):
                nc.sync.dma_start(w1_sb[:, fm], w1_re[:, fm])
            if nb > 1:
                dma_x(1)
            for dn in range(KD):
                nc.sync.dma_start(w2_sb[:, dn], w2_re[:, dn])
            for bi in range(2, nb):
                dma_x(bi)
            nc.sync.dma_start(cw_sb[:], cw_d[:])

            def mm1(bi):
                """z = 3-term x@w1 ; h8 = silu fp8 ; hf = silu f32 (per fm)."""
                b0, bs = blocks[bi]
                hfs = []
                x_sb = x_sbs[bi]
                for fm in range(KF):
                    ps = pshpool.tile([128, bs], f32, tag="psh",
                                      name=f"psh{bi}_{fm}")
                    n_i = KD + KD // 2
                    i = 0
                    for kb in range(KD):  # P1: (x_hi,x_hi)x(w_hi,w_lo)
                        nc.tensor.matmul(
                            ps[:],
                            lhsT=w1_sb[:, fm, kb],
                            rhs=x_sb[:, 0, kb].unsqueeze(1)
                                .broadcast_to([128, 2, bs]),
                            start=(i == 0), stop=(i == n_i - 1), perf_mode=DR)
                        i += 1
                    for kb2 in range(0, KD, 2):  # P2: (x_lo,x_lo)x(w_hi,w_hi)
                        nc.tensor.matmul(
                            ps[:],
                            lhsT=w1_sb[:, fm, kb2:kb2 + 2, 0],
                            rhs=x_sb[:, 1, kb2:kb2 + 2],
                            start=(i == 0), stop=(i == n_i - 1), perf_mode=DR)
                        i += 1
                    nc.scalar.activation(h_sb[:, fm, 0, b0:b0 + bs], ps[:],
                                         silu, scale=1.0 / (WS * XS))
                    hf = hfpool.tile([128, bs], f32, tag="hf",
                                     name=f"hf{bi}_{fm}")
                    nc.scalar.activation(hf[:], ps[:], silu,
                                         scale=1.0 / (WS * XS))
                    hfs.append(hf)
                return hfs

            def h_lo(bi, hfs):
                b0, bs = blocks[bi]
                for fm in range(KF):
                    nc.vector.scalar_tensor_tensor(
                        h_sb[:, fm, 1, b0:b0 + bs], hfs[fm][:], 1.0,
                        h_sb[:, fm, 0, b0:b0 + bs],
                        op0=mult, op1=subtract)

            def mm2(bi):
                """y = (3-term h@w2) * cw / WS ; per-dn SP (hw-DGE) DMAs."""
                b0, bs = blocks[bi]
                for dn in range(KD):
                    ps = psypool.tile([128, bs], f32, tag="psy",
                                      name=f"psy{bi}_{dn}")
                    n_i = KF + KF // 2
                    i = 0
                    for fb in range(KF):  # P1': (h8,h_lo)x(w2_hi,w2_hi)
                        nc.tensor.matmul(
                            ps[:],
                            lhsT=w2_sb[:, dn, fb, 0].unsqueeze(1)
                                .broadcast_to([128, 2, 128]),
                            rhs=h_sb[:, fb, :, b0:b0 + bs],
                            start=(i == 0), stop=(i == n_i - 1), perf_mode=DR)
                        i += 1
                    for fb2 in range(0, KF, 2):  # P2': (h8,h8)x(w2_lo,w2_lo)
                        nc.tensor.matmul(
                            ps[:],
                            lhsT=w2_sb[:, dn, fb2:fb2 + 2, 1],
                            rhs=h_sb[:, fb2:fb2 + 2, 0, b0:b0 + bs],
                            start=(i == 0), stop=(i == n_i - 1), perf_mode=DR)
                        i += 1
                    ys = ypool.tile([128, bs], bf16, tag="y",
                                    name=f"y{bi}_{dn}")
                    nc.vector.scalar_tensor_tensor(
                        ys[:], ps[:], 1.0 / WS, cw_sb[:, b0:b0 + bs],
                        op0=mult, op1=mult)
                    nc.sync.dma_start(y_re[:, dn, b0:b0 + bs], ys[:])

            # ---- software-pipelined emission: mm1 one block ahead of mm2
            hfs = mm1(0)
            h_lo(0, hfs)
            for bi in range(1, nb):
                hfs = mm1(bi)
                mm2(bi - 1)
                h_lo(bi, hfs)
            mm2(nb - 1)

    nc.compile()
    return nc


def _route(x: np.ndarray, gate_w: np.ndarray):
    """Router on host CPU with the reference's exact jax ops/dtypes."""
    try:
        import jax
        import jax.numpy as jnp
        with jax.default_device(jax.devices("cpu")[0]):
            logits = jnp.einsum('bsd,de->bse', jnp.asarray(x),
                                jnp.asarray(gate_w))
            top_logits, top_idx = jax.lax.top_k(logits, TOP_K)
            top_w = jax.nn.softmax(top_logits, axis=-1)
            ti = np.asarray(top_idx).reshape(T, TOP_K)
            tw = np.asarray(top_w).reshape(T, TOP_K).astype(np.float32)
    except Exception:
        # numpy fallback (same selection semantics as jax.lax.top_k)
        logits = (x.reshape(T, D_MODEL) @ gate_w).astype(np.float32)
        i0 = np.argmax(logits, axis=1)
        masked = logits.copy()
        masked[np.arange(T), i0] = -np.inf
        i1 = np.argmax(masked, axis=1)
        v0 = logits[np.arange(T), i0]
        v1 = logits[np.arange(T), i1]
        e1 = np.exp(v1 - v0)
        w0 = 1.0 / (1.0 + e1)
        ti = np.stack([i0, i1], 1)
        tw = np.stack([w0, 1.0 - w0], 1).astype(np.float32)
    return ti, tw


def _hi_lo(a: np.ndarray, F8):
    hi = a.astype(F8)
    lo = (a - hi.astype(np.float32)).astype(F8)
    return hi, lo


def kernel(x: np.ndarray, gate_w: np.ndarray, w1: np.ndarray,
           w2: np.ndarray) -> np.ndarray:
    from concourse.bass_utils import run_bass_kernel_spmd
    import ml_dtypes

    F8 = ml_dtypes.float8_e4m3

    x = np.asarray(x, dtype=np.float32)
    gate_w = np.asarray(gate_w, dtype=np.float32)
    w1 = np.asarray(w1, dtype=np.float32)
    w2 = np.asarray(w2, dtype=np.float32)

    ti, tw = _route(x, gate_w)

    x2d = x.reshape(T, D_MODEL)
    tokens, weights = [], []
    for e in range(N_EXPERTS):
        rows, ks = np.nonzero(ti == e)
        tokens.append(rows)
        weights.append(tw[rows, ks])
    counts = [len(t) for t in tokens]
    C = _round_up(max(max(counts), 512), 4)

    if C not in _PROGRAM_CACHE:
        _PROGRAM_CACHE[C] = _build_program(C)
    nc = _PROGRAM_CACHE[C]

    blocks = _blocks(C)
    in_maps = []
    for e in range(N_EXPERTS):
        n = counts[e]
        # x: [D, C] scaled by XS, hi/lo split, one tensor per token block
        # with row p = [v2, kb8, bs] (partition-contiguous chunks)
        xt = np.zeros((D_MODEL, C), dtype=np.float32)
        if n:
            xt[:, :n] = x2d[tokens[e]].T * XS
        x_hi, x_lo = _hi_lo(xt, F8)
        xs = np.stack([x_hi.reshape(KD, 128, C),
                       x_lo.reshape(KD, 128, C)])      # [v, kb, p, C]
        im = {}
        for bi, (b0, bs) in enumerate(blocks):
            im[f"xt{bi}"] = np.ascontiguousarray(
                xs[:, :, :, b0:b0 + bs].transpose(2, 0, 1, 3)
                ).reshape(128, 2 * KD * bs)

        # w1: row fm*128+p = [kb, v, f128]
        w1_hi, w1_lo = _hi_lo(w1[e] * WS, F8)          # [D, DFF]
        w1v = np.stack([w1_hi, w1_lo]).reshape(2, KD, 128, KF, 128)
        im["w1"] = np.ascontiguousarray(
            w1v.transpose(3, 2, 1, 0, 4)).reshape(KF * 128, KD * 2 * 128)

        # w2: row dn*128+p = [fb, v, d128]
        w2_hi, w2_lo = _hi_lo(w2[e] * WS, F8)          # [DFF, D]
        w2v = np.stack([w2_hi, w2_lo]).reshape(2, KF, 128, KD, 128)
        im["w2"] = np.ascontiguousarray(
            w2v.transpose(3, 2, 1, 0, 4)).reshape(KD * 128, KF * 2 * 128)

        cw = np.zeros((C,), dtype=np.float32)
        cw[:n] = weights[e]
        im["cw"] = np.broadcast_to(cw[None, :], (128, C)).copy()
        in_maps.append(im)

    res = run_bass_kernel_spmd(nc, in_maps, core_ids=list(range(N_CORES)))

    out2d = np.zeros((T, D_MODEL), dtype=np.float32)
    for e in range(N_EXPERTS):
        n = counts[e]
        if n:
            out2d[tokens[e]] += res.results[e]["y"].astype(np.float32).T[:n]

    LAST_BUILD["nc"] = nc
    LAST_BUILD["C"] = C
    return out2d.reshape(B, S, D_MODEL)
